# revision 1
# baseline (speedup 1.0000x reference)
"""Trainium2 Bass kernel for a pre-LN multi-head attention block.

Full-input contract: kernel(**inputs) takes the unsharded tensors from
setup_inputs() and returns the full [4, 2048, 1024] output.

Sharding: 8 cores = 4 batches x 2 head-groups (8 heads each).
Each core computes LayerNorm(x[b]) (replicated within the batch pair),
its 8 heads of QKV + attention, and a partial projection
(attn_out_part @ w_proj_rows).  Host sums the two partials per batch and
adds b_proj + residual.

Host-side algebraic folds (exact):
  - ln_w folded into w_qkv columns, ln_b folded into b_qkv
  - softmax scale (0.125, exact in fp32/bf16) folded into W_q / b_q

On-core dataflow (all layouts chosen so nothing but h needs transposing):
  LN:    x[128tok,1024] -> h bf16, PE-transpose -> hT [emb, tok]
  QKV:   QT/KT = (Wq|Wk)^T hT  -> [head_dim*2heads, tok] tiles
         V    = hT^T Wv (+ones-row bias matmul) -> V_aug [tok, 8x(64+1)]
  Attn:  ST[k,q] = KT^T-slice x QT-slice (contract d=64)
         expST = Exp(ST) on ACT -> bf16
         OT_aug[65, q] += V_aug^T @ expST   (row 64 = softmax sums)
         normalize: recip(ones^T@sums bcast) * OT -> OTn bf16
  Proj:  Z[tok, emb] = OTn^T-slices @ Wp rows, accumulated over 4 pairs
"""

import sys

sys.path.insert(0, "/opt/trn_rl_repo")

import numpy as np
import ml_dtypes

import concourse.bass as bass
from concourse import bacc
import concourse.tile as tile
from concourse import mybir
from concourse.bass_utils import run_bass_kernel_spmd
from concourse.masks import make_identity

EMB = 1024
HEADS = 16
HD = 64
SCALE = HD ** -0.5
N_TOK = 2048
N_CORES = 8
HPC = 8                 # heads per core
QK_COLS = HPC * HD      # 512
P = 128
NT = N_TOK // P         # 16 token tiles
EC = EMB // P           # 8 emb chunks
QCH = 4                 # q chunks of 512
NKT = 16                # k tiles of 128
NPAIR = HPC // 2        # 4 head-pair tiles

BF16 = mybir.dt.bfloat16
F32 = mybir.dt.float32
AF = mybir.ActivationFunctionType


def build_nc():
    nc = bacc.Bacc(trn_type="TRN2", target_bir_lowering=False)

    x_d = nc.dram_tensor("x", [N_TOK, EMB], F32, kind="ExternalInput")
    wq_d = nc.dram_tensor("wq", [EMB // 2, 2 * QK_COLS], BF16, kind="ExternalInput")
    wk_d = nc.dram_tensor("wk", [EMB // 2, 2 * QK_COLS], BF16, kind="ExternalInput")
    wv_d = nc.dram_tensor("wv", [EMB // 2, 2 * QK_COLS], BF16, kind="ExternalInput")
    bqt_d = nc.dram_tensor("bqt", [P, NPAIR], F32, kind="ExternalInput")
    bkt_d = nc.dram_tensor("bkt", [P, NPAIR], F32, kind="ExternalInput")
    bv_d = nc.dram_tensor("bv", [1, QK_COLS], BF16, kind="ExternalInput")
    wp_d = nc.dram_tensor("wp", [QK_COLS // 2, 2 * EMB], BF16, kind="ExternalInput")
    z_d = nc.dram_tensor("z", [N_TOK, EMB], F32, kind="ExternalOutput")

    with tile.TileContext(nc) as tc:
        _emit(nc, tc, x_d, wq_d, wk_d, wv_d, bqt_d, bkt_d, bv_d, wp_d, z_d)
    nc.finalize()
    return nc


def _emit(nc, tc, x_d, wq_d, wk_d, wv_d, bqt_d, bkt_d, bv_d, wp_d, z_d):
    from contextlib import ExitStack

    ctx = ExitStack()
    with ctx:
        consts = ctx.enter_context(tc.tile_pool(name="consts", bufs=1))
        persist = ctx.enter_context(tc.tile_pool(name="persist", bufs=1))

        ident = consts.tile([P, P], BF16, tag="ident", name="ident")
        make_identity(nc, ident)
        ones_row = consts.tile([1, P], BF16, tag="ones_row", name="ones_row")
        nc.vector.memset(ones_row, 1.0)
        ones64 = consts.tile([1, HD], BF16, tag="ones64", name="ones64")
        nc.vector.memset(ones64, 1.0)
        eps_t = consts.tile([P, 1], F32, tag="eps", name="eps")
        nc.vector.memset(eps_t, 1e-5)

        bqt = consts.tile([P, NPAIR], F32, tag="bqt", name="bqt")
        nc.sync.dma_start(out=bqt, in_=bqt_d[:, :])
        bkt = consts.tile([P, NPAIR], F32, tag="bkt", name="bkt")
        nc.sync.dma_start(out=bkt, in_=bkt_d[:, :])
        bvt = consts.tile([1, QK_COLS], BF16, tag="bvt", name="bvt")
        nc.sync.dma_start(out=bvt, in_=bv_d[:, :])

        wq_s = []
        wk_s = []
        wv_s = []
        for c in range(EC // 2):
            for lst, srcd, nm in ((wq_s, wq_d, "wq"), (wk_s, wk_d, "wk"),
                                  (wv_s, wv_d, "wv")):
                t = persist.tile([P, 2, QK_COLS], BF16, tag=f"{nm}{c}", name=f"{nm}{c}")
                nc.sync.dma_start(out=t, in_=srcd[c * P:(c + 1) * P, :].rearrange(
                    "p (r m) -> p r m", r=2))
                lst.append(t)
        wp_s = []
        for i in range(2):
            t = persist.tile([P, 2, EMB], BF16, tag=f"wp{i}", name=f"wp{i}")
            nc.sync.dma_start(out=t, in_=wp_d[i * P:(i + 1) * P, :].rearrange(
                "p (r m) -> p r m", r=2))
            wp_s.append(t)

        qt = [persist.tile([P, N_TOK], BF16, tag=f"qt{i}", name=f"qt{i}") for i in range(NPAIR)]
        kt = [persist.tile([P, N_TOK], BF16, tag=f"kt{i}", name=f"kt{i}") for i in range(NPAIR)]
        otn = [persist.tile([P, 2, N_TOK], BF16, tag=f"otn{i}", name=f"otn{i}") for i in range(2)]
        vaug = [persist.tile([P, 2, HPC, HD + 1], BF16, tag=f"vaug{i}", name=f"vaug{i}")
                for i in range(NT // 2)]
        for t in range(NT // 2):
            nc.vector.memset(vaug[t][:, :, :, HD:HD + 1], 1.0)

        # ---------------- Phase 1: LayerNorm + transpose ----------------
        ht_ctx = ExitStack()
        ht_pool = ht_ctx.enter_context(tc.tile_pool(name="ht", bufs=1))
        ht = [ht_pool.tile([P, 2, N_TOK], BF16, tag=f"ht{e}", name=f"ht{e}") for e in range(EC // 2)]

        with tc.tile_pool(name="ln", bufs=3) as ln_pool, \
             tc.tile_pool(name="lns", bufs=6) as lns, \
             tc.tile_pool(name="ps_tr", bufs=3, space="PSUM") as ps_tr:
            for t in range(NT):
                x_t = ln_pool.tile([P, EMB], F32, tag="x", name="x")
                nc.sync.dma_start(out=x_t, in_=x_d[t * P:(t + 1) * P, :])
                stats = lns.tile([P, 2, 6], F32, tag="stats", name="stats")
                nc.vector.bn_stats(out=stats[:, 0, :], in_=x_t[:, 0:512])
                nc.vector.bn_stats(out=stats[:, 1, :], in_=x_t[:, 512:1024])
                mv = lns.tile([P, 2], F32, tag="mv", name="mv")
                nc.vector.bn_aggr(out=mv, in_=stats)
                sd = lns.tile([P, 1], F32, tag="sd", name="sd")
                nc.scalar.activation(out=sd, in_=mv[:, 1:2], func=AF.Sqrt,
                                     bias=eps_t, scale=1.0)
                rstd = lns.tile([P, 1], F32, tag="rstd", name="rstd")
                nc.vector.reciprocal(out=rstd, in_=sd)
                nmean = lns.tile([P, 1], F32, tag="nmean", name="nmean")
                nc.vector.tensor_scalar_mul(nmean, mv[:, 0:1], -1.0)
                xc = ln_pool.tile([P, EMB], F32, tag="xc", name="xc")
                nc.vector.tensor_scalar_add(xc, x_t, nmean)
                h_t = ln_pool.tile([P, EMB], BF16, tag="h", name="h")
                nc.vector.tensor_scalar_mul(h_t, xc, rstd)
                for e in range(EC):
                    pt = ps_tr.tile([P, P], BF16, tag="tr", name="tr")
                    nc.tensor.transpose(pt, h_t[:, e * P:(e + 1) * P], ident)
                    nc.vector.tensor_copy(
                        out=ht[e // 2][:, e % 2, t * P:(t + 1) * P], in_=pt)

        # ---------------- Phase 2: QKV matmuls ----------------
        with tc.tile_pool(name="ps_qkv", bufs=3, space="PSUM") as ps_qkv:
            for m in range(NPAIR):
                for n in range(QCH):
                    pq = ps_qkv.tile([P, 512], F32, tag="qkv", name="qkv")
                    for c in range(EC // 2):
                      for r in range(2):
                        nc.tensor.matmul(pq, lhsT=wq_s[c][:, r, m * P:(m + 1) * P],
                                         rhs=ht[c][:, r, n * 512:(n + 1) * 512],
                                         start=(c == 0 and r == 0),
                                         stop=(c == EC // 2 - 1 and r == 1))
                    nc.scalar.activation(out=qt[m][:, n * 512:(n + 1) * 512],
                                         in_=pq, func=AF.Identity,
                                         bias=bqt[:, m:m + 1], scale=1.0)
                    pk = ps_qkv.tile([P, 512], F32, tag="qkv", name="qkv")
                    for c in range(EC // 2):
                      for r in range(2):
                        nc.tensor.matmul(pk, lhsT=wk_s[c][:, r, m * P:(m + 1) * P],
                                         rhs=ht[c][:, r, n * 512:(n + 1) * 512],
                                         start=(c == 0 and r == 0),
                                         stop=(c == EC // 2 - 1 and r == 1))
                    nc.scalar.activation(out=kt[m][:, n * 512:(n + 1) * 512],
                                         in_=pk, func=AF.Identity,
                                         bias=bkt[:, m:m + 1], scale=1.0)
            for t in range(NT):
                pv = ps_qkv.tile([P, 512], F32, tag="qkv", name="qkv")
                for c in range(EC // 2):
                  for r in range(2):
                    nc.tensor.matmul(pv, lhsT=ht[c][:, r, t * P:(t + 1) * P],
                                     rhs=wv_s[c][:, r, :],
                                     start=(c == 0 and r == 0), stop=False)
                nc.tensor.matmul(pv, lhsT=ones_row, rhs=bvt,
                                 start=False, stop=True)
                nc.vector.tensor_copy(
                    out=vaug[t // 2][:, t % 2, :, 0:HD],
                    in_=pv.rearrange("p (h d) -> p h d", h=HPC))

        ht_ctx.close()

        # ---------------- Phase 3: attention ----------------
        with tc.tile_pool(name="expp", bufs=20) as expp, \
             tc.tile_pool(name="att_sm", bufs=4) as att_sm, \
             tc.tile_pool(name="ps_st", bufs=2, space="PSUM") as ps_st, \
             tc.tile_pool(name="ps_ot", bufs=2, space="PSUM") as ps_ot, \
             tc.tile_pool(name="ps_b", bufs=1, space="PSUM") as ps_b:
            def emit_st(h, q):
                """Scores + exp for one (head, q-chunk); returns exp tiles."""
                pair, row = divmod(h, 2)
                row *= HD
                ets = []
                for c in range(NKT // 2):
                    pst = ps_st.tile([P, 2, 512], F32, tag="st", name="st")
                    for r in range(2):
                        k = 2 * c + r
                        nc.tensor.matmul(
                            pst[:, r, :],
                            lhsT=kt[pair][row:row + HD, k * P:(k + 1) * P],
                            rhs=qt[pair][row:row + HD, q * 512:(q + 1) * 512],
                            start=True, stop=True)
                    e_t = expp.tile([P, 2, 512], BF16, tag="e", name="e")
                    nc.scalar.activation(out=e_t, in_=pst, func=AF.Exp)
                    ets.append(e_t)
                return ets

            def emit_av(h, q, ets):
                """att@v + normalize for one (head, q-chunk)."""
                pot = ps_ot.tile([HD + 1, 512], F32, tag="ot", name="ot")
                for k in range(NKT):
                    nc.tensor.matmul(pot, lhsT=vaug[k // 2][:, k % 2, h, :],
                                     rhs=ets[k // 2][:, k % 2, :],
                                     start=(k == 0), stop=(k == NKT - 1))
                srow = att_sm.tile([1, 512], BF16, tag="srow", name="srow")
                nc.vector.tensor_copy(out=srow, in_=pot[HD:HD + 1, :])
                pb = ps_b.tile([HD, 512], F32, tag="b", name="b")
                nc.tensor.matmul(pb, lhsT=ones64, rhs=srow,
                                 start=True, stop=True)
                rec = att_sm.tile([HD, 512], F32, tag="rec", name="rec")
                nc.vector.reciprocal(out=rec, in_=pb)
                nc.vector.tensor_mul(
                    otn[h // 4][(h % 2) * HD:(h % 2) * HD + HD, (h // 2) % 2,
                                q * 512:(q + 1) * 512],
                    pot[0:HD, :], rec)

            prev = None
            for h in range(HPC):
                for q in range(QCH):
                    ets = emit_st(h, q)
                    if prev is not None:
                        emit_av(*prev)
                    prev = (h, q, ets)
            emit_av(*prev)

        # ---------------- Phase 4: projection ----------------
        with tc.tile_pool(name="ps_z", bufs=3, space="PSUM") as ps_z, \
             tc.tile_pool(name="zst", bufs=3) as zst:
            for t in range(NT):
                for ec2 in range(2):
                    pz = ps_z.tile([P, 512], F32, tag="z", name="z")
                    for c in range(2):
                      for r in range(2):
                        nc.tensor.matmul(
                            pz, lhsT=otn[c][:, r, t * P:(t + 1) * P],
                            rhs=wp_s[c][:, r, ec2 * 512:(ec2 + 1) * 512],
                            start=(c == 0 and r == 0), stop=(c == 1 and r == 1))
                    z_t = zst.tile([P, 512], F32, tag="z", name="z")
                    nc.vector.tensor_copy(out=z_t, in_=pz)
                    nc.sync.dma_start(
                        out=z_d[t * P:(t + 1) * P, ec2 * 512:(ec2 + 1) * 512],
                        in_=z_t)


_CACHE = {}


def _get_nc():
    if "nc" not in _CACHE:
        _CACHE["nc"] = build_nc()
    return _CACHE["nc"]


def _prep_in_maps(x, ln_w, ln_b, w_qkv, b_qkv, w_proj, b_proj):
    bf = ml_dtypes.bfloat16
    x = np.asarray(x, np.float32)
    ln_w = np.asarray(ln_w, np.float32)
    ln_b = np.asarray(ln_b, np.float32)
    w_qkv = np.asarray(w_qkv, np.float32)
    b_qkv = np.asarray(b_qkv, np.float32)
    w_proj = np.asarray(w_proj, np.float32)

    b_eff = b_qkv + ln_b @ w_qkv
    w_eff = ln_w[:, None] * w_qkv
    w4 = w_eff.reshape(EMB, HEADS, HD, 3)
    b4 = b_eff.reshape(HEADS, HD, 3)
    wq = w4[..., 0] * SCALE
    wk = w4[..., 1]
    wv = w4[..., 2]
    bq = b4[..., 0] * SCALE
    bk = b4[..., 1]
    bv = b4[..., 2]

    def _dr(w):
        # [R, M] -> [R/2, 2M]: row 256c+128r+k -> (c*128+k, r*M+m)
        R, M = w.shape
        return np.ascontiguousarray(
            w.reshape(R // 256, 2, 128, M).transpose(0, 2, 1, 3).reshape(R // 2, 2 * M))

    def bias_t(b, hsl):
        # [8, 64] head-slice -> [512] (pair-major) -> [128, 4] transposed
        v = b[hsl].reshape(QK_COLS)
        return np.ascontiguousarray(v.reshape(NPAIR, P).T.astype(np.float32))

    in_maps = []
    for cid in range(N_CORES):
        bi, hg = divmod(cid, 2)
        hsl = slice(hg * HPC, (hg + 1) * HPC)
        in_maps.append({
            "x": np.ascontiguousarray(x[bi]),
            "wq": _dr(wq[:, hsl, :].reshape(EMB, QK_COLS)).astype(bf),
            "wk": _dr(wk[:, hsl, :].reshape(EMB, QK_COLS)).astype(bf),
            "wv": _dr(wv[:, hsl, :].reshape(EMB, QK_COLS)).astype(bf),
            "bqt": bias_t(bq, hsl),
            "bkt": bias_t(bk, hsl),
            "bv": np.ascontiguousarray(
                bv[hsl].reshape(1, QK_COLS)).astype(bf),
            "wp": _dr(w_proj[hg * QK_COLS:(hg + 1) * QK_COLS, :]).astype(bf),
        })
    return in_maps


def _gather(results, x, b_proj):
    b_proj = np.asarray(b_proj, np.float32)
    x = np.asarray(x, np.float32)
    out = np.empty((x.shape[0], N_TOK, EMB), np.float32)
    for bi in range(x.shape[0]):
        out[bi] = (results[2 * bi]["z"] + results[2 * bi + 1]["z"]
                   + b_proj[None, :] + x[bi])
    return out


def _run(inputs, **kw):
    in_maps = _prep_in_maps(**inputs)
    res = run_bass_kernel_spmd(_get_nc(), in_maps,
                               core_ids=list(range(N_CORES)), **kw)
    out = _gather(res.results, inputs["x"], inputs["b_proj"])
    return out, res


def kernel(**inputs):
    out, _ = _run(inputs)
    return out



# revision 28
# speedup vs baseline: 1.0858x; 1.0858x over previous
"""Trainium2 Bass kernel for a pre-LN multi-head attention block.

Full-input contract: kernel(**inputs) takes the unsharded tensors from
setup_inputs() and returns the full [4, 2048, 1024] output.

Sharding: 8 cores = 4 batches x 2 head-groups (8 heads each).
Each core computes LayerNorm(x[b]), its 8 heads of QKV + attention, and a
partial projection.  Host sums the two partials per batch and adds
b_proj_eff + residual.

Numerics strategy: all matmuls run in fp8e4 (e4m3) DoubleRow perf mode
(2 contraction sub-rows per instruction at 0.5 cycles/row).  Weights are
pre-scaled x32 into fp8's representable range host-side; the 1/32 is
folded into the PSUM->SBUF cast passes.  The attention path is only ~8%
of the output norm (residual dominates), so fp8's ~6% element error
lands at ~2e-3 total relative error, well under tolerance.

Host-side algebraic folds (exact):
  - ln_w folded into w_qkv columns, ln_b folded into b_qkv
  - softmax scale folded into W_q / b_q
  - V bias folded into b_proj (attention rows sum to 1)

Layouts (DoubleRow pairs contraction rows along a middle dim of 2):
  ht    [128, 8, 2048] fp8: ht[p, 2c+i, t] = h[t, emb=256c+128i+p]
  qt/kt [g][128, 2, 2048] fp8: [32s+j, r, t] = Q[t, head 4g+s, d=32r+j]
  vaug  [kk][128, 2, 8, 65] fp8: [p, i, h, d] = V[tok=256kk+128i+p, h, d],
        d=64 column = 1.0 (softmax-sum row trick)
  E     [128, 2, 512] fp8 per (k-pair, q-chunk): [p, i, q]=exp(S[k,q])
  otn   [g][128, 2, 2048] fp8: [p, i, t] = 8*OT_norm[hd=256g+128i+p, t]
"""

import sys

sys.path.insert(0, "/opt/trn_rl_repo")

import numpy as np
import ml_dtypes

import concourse.bass as bass
from concourse import bacc
import concourse.tile as tile
from concourse import mybir
from concourse.bass_utils import run_bass_kernel_spmd
from concourse.masks import make_identity

EMB = 1024
HEADS = 16
HD = 64
SCALE = HD ** -0.5
N_TOK = 2048
N_CORES = 8
HPC = 8                 # heads per core
QK_COLS = HPC * HD      # 512
P = 128
NT = N_TOK // P         # 16 token tiles
QCH = 4                 # q chunks of 512
NKT = 16                # k tiles of 128
WSCALE = 32.0           # host pre-scale on all weights (fp8 range)
OSCALE = 8.0            # extra scale on normalized attn output

BF16 = mybir.dt.bfloat16
F32 = mybir.dt.float32
FP8 = mybir.dt.float8e4
AF = mybir.ActivationFunctionType
DR = mybir.MatmulPerfMode.DoubleRow


def build_nc():
    nc = bacc.Bacc(trn_type="TRN2", target_bir_lowering=False)

    x_d = nc.dram_tensor("x", [N_TOK, EMB], F32, kind="ExternalInput")
    wq_d = nc.dram_tensor("wq", [P, 2 * 8 * 4 * HD], FP8, kind="ExternalInput")
    wk_d = nc.dram_tensor("wk", [P, 2 * 8 * 4 * HD], FP8, kind="ExternalInput")
    wv_d = nc.dram_tensor("wv", [P, 2 * 4 * 512], FP8, kind="ExternalInput")
    wp_d = nc.dram_tensor("wp", [P, 2 * 2 * EMB], FP8, kind="ExternalInput")
    bqt_d = nc.dram_tensor("bqt", [HD, HPC], F32, kind="ExternalInput")
    bkt_d = nc.dram_tensor("bkt", [HD, HPC], F32, kind="ExternalInput")
    z_d = nc.dram_tensor("z", [N_TOK, EMB], BF16, kind="ExternalOutput")

    with tile.TileContext(nc) as tc:
        _emit(nc, tc, x_d, wq_d, wk_d, wv_d, wp_d, bqt_d, bkt_d, z_d)
    nc.finalize()
    return nc


def _emit(nc, tc, x_d, wq_d, wk_d, wv_d, wp_d, bqt_d, bkt_d, z_d):
    from contextlib import ExitStack

    ctx = ExitStack()
    with ctx:
        consts = ctx.enter_context(tc.tile_pool(name="consts", bufs=1))
        persist = ctx.enter_context(tc.tile_pool(name="persist", bufs=1))

        ident = consts.tile([P, P], BF16, tag="ident", name="ident")
        make_identity(nc, ident)
        ones64 = consts.tile([1, HD], BF16, tag="ones64", name="ones64")
        nc.vector.memset(ones64, OSCALE)
        eps_t = consts.tile([P, 1], F32, tag="eps", name="eps")
        nc.vector.memset(eps_t, 1e-5)
        nshift = consts.tile([P, 1], F32, tag="nshift", name="nshift")
        nc.vector.memset(nshift, -6.0)

        # weights: wq/wk as [128, 2(i), 8(h), 4(c), 64(m=d)]
        wq_s = persist.tile([P, 2, 8, 4, HD], FP8, tag="wq", name="wq")
        nc.sync.dma_start(out=wq_s, in_=wq_d[:, :].rearrange(
            "p (i h c m) -> p i h c m", i=2, h=8, c=4))
        wk_s = persist.tile([P, 2, 8, 4, HD], FP8, tag="wk", name="wk")
        nc.sync.dma_start(out=wk_s, in_=wk_d[:, :].rearrange(
            "p (i h c m) -> p i h c m", i=2, h=8, c=4))
        # wv as [128, 2(i), 4(c), 512(hd)]
        wv_s = persist.tile([P, 2, 4, 512], FP8, tag="wv", name="wv")
        nc.sync.dma_start(out=wv_s, in_=wv_d[:, :].rearrange(
            "p (i c m) -> p i c m", i=2, c=4))
        # wp as [128, 2(i), 2(g), 1024(n)]
        wp_s = persist.tile([P, 2, 2, EMB], FP8, tag="wp", name="wp")
        nc.sync.dma_start(out=wp_s, in_=wp_d[:, :].rearrange(
            "p (i g n) -> p i g n", i=2, g=2))
        bqt = consts.tile([HD, HPC], F32, tag="bqt", name="bqt")
        nc.sync.dma_start(out=bqt, in_=bqt_d[:, :])
        bkt = consts.tile([HD, HPC], F32, tag="bkt", name="bkt")
        nc.sync.dma_start(out=bkt, in_=bkt_d[:, :])

        ht = persist.tile([P, 8, N_TOK], FP8, tag="ht", name="ht")
        # per-head Q^T/K^T at base partition 0 (DoubleRow needs row pos 0);
        # DR slice i=1 is zeros (contraction zero-pad, free-dim cost model)
        qt = persist.tile([HD, HPC, 2, N_TOK], FP8, tag="qt", name="qt")
        kt = persist.tile([HD, HPC, 2, N_TOK], FP8, tag="kt", name="kt")
        nc.vector.memset(qt[:, :, 1, :], 0.0)
        nc.vector.memset(kt[:, :, 1, :], 0.0)
        otn = [persist.tile([P, 2, N_TOK], FP8, tag=f"otn{g}", name=f"otn{g}")
               for g in range(2)]
        # head stride 66 keeps the DR slice stride (8*66=528B) 16B-aligned
        vaug = [persist.tile([P, 2, HPC, HD + 2], FP8, tag=f"va{k}", name=f"va{k}")
                for k in range(NKT // 2)]
        for k in range(NKT // 2):
            nc.vector.memset(vaug[k][:, :, :, HD:HD + 1], 1.0)

        # ---------------- Phase 1: LayerNorm + transpose ----------------
        with tc.tile_pool(name="ln", bufs=3) as ln_pool, \
             tc.tile_pool(name="lns", bufs=6) as lns, \
             tc.tile_pool(name="ps_a", bufs=2, space="PSUM") as ps_a:
            for t in range(NT):
                x_t = ln_pool.tile([P, EMB], F32, tag="x", name="x")
                nc.sync.dma_start(out=x_t, in_=x_d[t * P:(t + 1) * P, :])
                stats = lns.tile([P, 2, 6], F32, tag="stats", name="stats")
                nc.vector.bn_stats(out=stats[:, 0, :], in_=x_t[:, 0:512])
                nc.vector.bn_stats(out=stats[:, 1, :], in_=x_t[:, 512:1024])
                mv = lns.tile([P, 2], F32, tag="mv", name="mv")
                nc.vector.bn_aggr(out=mv, in_=stats)
                sd = lns.tile([P, 1], F32, tag="sd", name="sd")
                nc.scalar.activation(out=sd, in_=mv[:, 1:2], func=AF.Sqrt,
                                     bias=eps_t, scale=1.0)
                rstd = lns.tile([P, 1], F32, tag="rstd", name="rstd")
                nc.vector.reciprocal(out=rstd, in_=sd)
                nms = lns.tile([P, 1], F32, tag="nms", name="nms")
                nc.vector.scalar_tensor_tensor(
                    out=nms, in0=mv[:, 0:1], scalar=-1.0, in1=rstd,
                    op0=mybir.AluOpType.mult, op1=mybir.AluOpType.mult)
                h_t = ln_pool.tile([P, EMB], BF16, tag="h", name="h")
                nc.scalar.activation(out=h_t, in_=x_t, func=AF.Identity,
                                     bias=nms, scale=rstd)
                pt = ps_a.tile([P, 8, P], BF16, tag="tr", name="tr")
                for j in range(8):
                    nc.tensor.transpose(pt[:, j, :], h_t[:, j * P:(j + 1) * P],
                                        ident)
                nc.vector.tensor_copy(out=ht[:, :, t * P:(t + 1) * P], in_=pt)

        # ---------------- Phase 2: QKV (fp8 DoubleRow) ----------------
        with tc.tile_pool(name="ps_qkv", bufs=3, space="PSUM") as ps_qkv:
            for h in range(HPC):
                for n in range(QCH):
                    pq = ps_qkv.tile([HD, 512], F32, tag="qkv", name="qkv")
                    for c in range(4):
                        nc.tensor.matmul(
                            pq, lhsT=wq_s[:, :, h, c, :],
                            rhs=ht[:, 2 * c:2 * c + 2, n * 512:(n + 1) * 512],
                            start=(c == 0), stop=(c == 3), perf_mode=DR)
                    nc.vector.tensor_scalar(
                        out=qt[:, h, 0, n * 512:(n + 1) * 512], in0=pq,
                        scalar1=1.0 / WSCALE, scalar2=bqt[:, h:h + 1],
                        op0=mybir.AluOpType.mult, op1=mybir.AluOpType.add)
                    pk = ps_qkv.tile([HD, 512], F32, tag="qkv", name="qkv")
                    for c in range(4):
                        nc.tensor.matmul(
                            pk, lhsT=wk_s[:, :, h, c, :],
                            rhs=ht[:, 2 * c:2 * c + 2, n * 512:(n + 1) * 512],
                            start=(c == 0), stop=(c == 3), perf_mode=DR)
                    nc.vector.tensor_scalar(
                        out=kt[:, h, 0, n * 512:(n + 1) * 512], in0=pk,
                        scalar1=1.0 / WSCALE, scalar2=bkt[:, h:h + 1],
                        op0=mybir.AluOpType.mult, op1=mybir.AluOpType.add)
            for t in range(NT):
                pv = ps_qkv.tile([P, 512], F32, tag="qkv", name="qkv")
                for c in range(4):
                    nc.tensor.matmul(
                        pv, lhsT=ht[:, 2 * c:2 * c + 2, t * P:(t + 1) * P],
                        rhs=wv_s[:, :, c, :],
                        start=(c == 0), stop=(c == 3), perf_mode=DR)
                nc.vector.tensor_scalar_mul(
                    vaug[t // 2][:, t % 2, :, 0:HD],
                    pv.rearrange("p (h d) -> p h d", h=HPC), 1.0 / WSCALE)

        # ---------------- Phase 3+4: attention + projection ----------------
        with tc.tile_pool(name="expp", bufs=20) as expp, \
             tc.tile_pool(name="att_sm", bufs=6) as att_sm, \
             tc.tile_pool(name="zst", bufs=4) as zst, \
             tc.tile_pool(name="ps_st", bufs=2, space="PSUM") as ps_st, \
             tc.tile_pool(name="ps_ot", bufs=2, space="PSUM") as ps_ot, \
             tc.tile_pool(name="ps_z", bufs=2, space="PSUM") as ps_z:

            def emit_st(h, q):
                """Scores + exp for one (head, q-chunk); returns E tiles."""
                ets = []
                for c in range(NKT // 2):
                    pst = ps_st.tile([P, 2, 512], F32, tag="st", name="st")
                    for i in range(2):
                        k = 2 * c + i
                        nc.tensor.matmul(
                            pst[:, i, :],
                            lhsT=kt[:, h, :, k * P:(k + 1) * P],
                            rhs=qt[:, h, :, q * 512:(q + 1) * 512],
                            start=True, stop=True, perf_mode=DR)
                    e_t = expp.tile([P, 2, 512], FP8, tag="e", name="e")
                    # shift by -6 so exp fits fp8e4 range (cancels in softmax)
                    nc.scalar.activation(out=e_t, in_=pst, func=AF.Exp,
                                         bias=nshift)
                    ets.append(e_t)
                return ets

            def emit_av(h, q, ets):
                """att@v + normalize for one (head, q-chunk)."""
                g = h // 4
                pot = ps_ot.tile([P, 512], F32, tag="ot", name="ot")
                for c in range(NKT // 2):
                    nc.tensor.matmul(pot[0:HD + 1, :],
                                     lhsT=vaug[c][:, :, h, 0:HD + 1],
                                     rhs=ets[c],
                                     start=(c == 0), stop=(c == NKT // 2 - 1),
                                     perf_mode=DR)
                rec1 = att_sm.tile([1, 512], BF16, tag="rec", name="rec")
                with nc.allow_low_precision(reason="softmax recip; bf16 ample"):
                    nc.vector.reciprocal(out=rec1, in_=pot[HD:HD + 1, :])
                nc.tensor.matmul(pot[HD:P, :], lhsT=ones64, rhs=rec1,
                                 start=True, stop=True, skip_group_check=True)
                pb_sb = att_sm.tile([HD, 512], BF16, tag="pb", name="pb")
                nc.vector.tensor_copy(out=pb_sb, in_=pot[HD:P, :])
                nc.vector.tensor_mul(
                    otn[g][64 * (h % 2):64 * (h % 2) + 64, (h % 4) // 2,
                           q * 512:(q + 1) * 512],
                    pot[0:HD, :], pb_sb)

            def emit_proj(t):
                """projection for one token tile (128 tokens x 1024 emb)."""
                for ec in range(2):
                    pz = ps_z.tile([P, 512], F32, tag="z", name="z")
                    for g in range(2):
                        nc.tensor.matmul(
                            pz, lhsT=otn[g][:, :, t * P:(t + 1) * P],
                            rhs=wp_s[:, :, g, ec * 512:(ec + 1) * 512],
                            start=(g == 0), stop=(g == 1), perf_mode=DR)
                    z_t = zst.tile([P, 512], BF16, tag="z", name="z")
                    nc.vector.tensor_scalar_mul(z_t, pz, 1.0 / (WSCALE * OSCALE))
                    nc.sync.dma_start(
                        out=z_d[t * P:(t + 1) * P, ec * 512:(ec + 1) * 512],
                        in_=z_t)

            prev = None
            for q in range(QCH):
                for h in range(HPC):
                    ets = emit_st(h, q)
                    if prev is not None:
                        emit_av(*prev)
                    prev = (h, q, ets)
                if q > 0:
                    for t in range(4 * (q - 1), 4 * q):
                        emit_proj(t)
            emit_av(*prev)
            for t in range(4 * (QCH - 1), NT):
                emit_proj(t)


_CACHE = {}


def _get_nc():
    if "nc" not in _CACHE:
        _CACHE["nc"] = build_nc()
    return _CACHE["nc"]


def _prep_in_maps(x, ln_w, ln_b, w_qkv, b_qkv, w_proj, b_proj):
    bf = ml_dtypes.bfloat16
    f8 = ml_dtypes.float8_e4m3fn
    x = np.asarray(x, np.float32)
    ln_w = np.asarray(ln_w, np.float32)
    ln_b = np.asarray(ln_b, np.float32)
    w_qkv = np.asarray(w_qkv, np.float32)
    b_qkv = np.asarray(b_qkv, np.float32)
    w_proj = np.asarray(w_proj, np.float32)
    b_proj = np.asarray(b_proj, np.float32)

    b_eff = b_qkv + ln_b @ w_qkv
    w_eff = ln_w[:, None] * w_qkv
    w4 = w_eff.reshape(EMB, HEADS, HD, 3)
    b4 = b_eff.reshape(HEADS, HD, 3)
    wq = w4[..., 0] * SCALE
    wk = w4[..., 1]
    wv = w4[..., 2]
    bq = b4[..., 0] * SCALE
    bk = b4[..., 1]
    bv = b4[..., 2]

    def pack_qk(w, hsl):
        # w [EMB, 8 heads, 64] -> [128p, 2i, 8h, 4c, 64d] fp8 (x WSCALE)
        # emb = 256c + 128i + p
        wh = w[:, hsl, :]                                    # [1024, 8, 64]
        wh = wh.reshape(4, 2, P, HPC, HD)                    # c i p h d
        wh = wh.transpose(2, 1, 3, 0, 4)                     # p i h c d
        return np.ascontiguousarray((wh * WSCALE).reshape(P, -1)).astype(f8)

    def pack_qk_bias(b, hsl):
        # b [8 heads, 64] -> [64d, 8h] f32
        return np.ascontiguousarray(b[hsl].T.astype(np.float32))

    def pack_v(w, hsl):
        # w [EMB, 8, 64] -> [128p, 2i, 4c, 512hd] fp8 (x WSCALE)
        wh = w[:, hsl, :].reshape(4, 2, P, 512)              # c i p hd
        wh = wh.transpose(2, 1, 0, 3)                        # p i c hd
        return np.ascontiguousarray((wh * WSCALE).reshape(P, -1)).astype(f8)

    def pack_wp(w, hg):
        # w_proj rows for this head group [512, 1024] -> [128p, 2i, 2g, 1024]
        wh = w[hg * 512:(hg + 1) * 512, :]                   # hd=256g+128i+p
        wh = wh.reshape(2, 2, P, EMB).transpose(2, 1, 0, 3)  # p i g n
        return np.ascontiguousarray((wh * WSCALE).reshape(P, -1)).astype(f8)

    in_maps = []
    for cid in range(N_CORES):
        bi, hg = divmod(cid, 2)
        hsl = slice(hg * HPC, (hg + 1) * HPC)
        in_maps.append({
            "x": np.ascontiguousarray(x[bi]),
            "wq": pack_qk(wq, hsl),
            "wk": pack_qk(wk, hsl),
            "wv": pack_v(wv, hsl),
            "wp": pack_wp(w_proj, hg),
            "bqt": pack_qk_bias(bq, hsl),
            "bkt": pack_qk_bias(bk, hsl),
        })
    return in_maps


def _gather(results, x, b_proj_eff):
    x = np.asarray(x, np.float32)
    out = np.empty((x.shape[0], N_TOK, EMB), np.float32)
    for bi in range(x.shape[0]):
        out[bi] = (results[2 * bi]["z"].astype(np.float32)
                   + results[2 * bi + 1]["z"].astype(np.float32)
                   + b_proj_eff[None, :] + x[bi])
    return out


def _run(inputs, **kw):
    in_maps = _prep_in_maps(**inputs)
    # exact fold of V bias into projection bias
    b_eff = np.asarray(inputs["b_qkv"], np.float32) + \
        np.asarray(inputs["ln_b"], np.float32) @ np.asarray(
            inputs["w_qkv"], np.float32)
    bv = b_eff.reshape(HEADS, HD, 3)[..., 2].reshape(HEADS * HD)
    b_proj_eff = np.asarray(inputs["b_proj"], np.float32) + \
        bv @ np.asarray(inputs["w_proj"], np.float32)
    res = run_bass_kernel_spmd(_get_nc(), in_maps,
                               core_ids=list(range(N_CORES)), **kw)
    out = _gather(res.results, inputs["x"], b_proj_eff)
    return out, res


def kernel(**inputs):
    out, _ = _run(inputs)
    return out


# revision 38
# speedup vs baseline: 1.3539x; 1.2469x over previous
"""Trainium2 Bass kernel for a pre-LN multi-head attention block.

Full-input contract: kernel(**inputs) takes the unsharded tensors from
setup_inputs() and returns the full [4, 2048, 1024] output.

Sharding: 8 cores = 4 batches x 2 head-groups (8 heads each).
Each core computes LayerNorm(x[b]), its 8 heads of QKV + attention, and a
partial projection.  Host sums the two partials per batch and adds
b_proj_eff + residual.

Numerics strategy: all matmuls run in fp8e4 (e4m3) DoubleRow perf mode
(2 contraction sub-rows per instruction at 0.5 cycles/row).  Weights are
pre-scaled x32 into fp8's representable range host-side; the 1/32 is
folded into the PSUM->SBUF cast passes.  The attention path is only ~8%
of the output norm (residual dominates), so fp8's ~6% element error
lands at ~2e-3 total relative error, well under tolerance.

Host-side algebraic folds (exact):
  - ln_w folded into w_qkv columns, ln_b folded into b_qkv
  - softmax scale folded into W_q / b_q
  - V bias folded into b_proj (attention rows sum to 1)

Layouts (DoubleRow pairs contraction rows along a middle dim of 2):
  ht    [128, 8, 2048] fp8: ht[p, 2c+i, t] = h[t, emb=256c+128i+p]
  qt/kt [g][128, 2, 2048] fp8: [32s+j, r, t] = Q[t, head 4g+s, d=32r+j]
  vaug  [kk][128, 2, 8, 65] fp8: [p, i, h, d] = V[tok=256kk+128i+p, h, d],
        d=64 column = 1.0 (softmax-sum row trick)
  E     [128, 2, 512] fp8 per (k-pair, q-chunk): [p, i, q]=exp(S[k,q])
  otn   [g][128, 2, 2048] fp8: [p, i, t] = 8*OT_norm[hd=256g+128i+p, t]
"""

import sys

sys.path.insert(0, "/opt/trn_rl_repo")

import numpy as np
import ml_dtypes

import concourse.bass as bass
from concourse import bacc
import concourse.tile as tile
from concourse import mybir
from concourse.bass_utils import run_bass_kernel_spmd
from concourse.masks import make_identity

EMB = 1024
HEADS = 16
HD = 64
SCALE = HD ** -0.5
N_TOK = 2048
N_CORES = 8
HPC = 8                 # heads per core
QK_COLS = HPC * HD      # 512
P = 128
NT = N_TOK // P         # 16 token tiles
QCH = 4                 # q chunks of 512
NKT = 16                # k tiles of 128
WSCALE = 32.0           # host pre-scale on all weights (fp8 range)
OSCALE = 8.0            # extra scale on normalized attn output

BF16 = mybir.dt.bfloat16
F32 = mybir.dt.float32
FP8 = mybir.dt.float8e4
AF = mybir.ActivationFunctionType
DR = mybir.MatmulPerfMode.DoubleRow


def build_nc():
    nc = bacc.Bacc(trn_type="TRN2", target_bir_lowering=False)

    x_d = nc.dram_tensor("x", [N_TOK, EMB], F32, kind="ExternalInput")
    wq_d = nc.dram_tensor("wq", [P, 2 * 8 * 4 * HD], FP8, kind="ExternalInput")
    wk_d = nc.dram_tensor("wk", [P, 2 * 8 * 4 * HD], FP8, kind="ExternalInput")
    wv_d = nc.dram_tensor("wv", [P, 2 * 4 * 512], FP8, kind="ExternalInput")
    wp_d = nc.dram_tensor("wp", [P, 2 * 2 * EMB], FP8, kind="ExternalInput")
    bqt_d = nc.dram_tensor("bqt", [HD, HPC], F32, kind="ExternalInput")
    bkt_d = nc.dram_tensor("bkt", [HD, HPC], F32, kind="ExternalInput")
    z_d = nc.dram_tensor("z", [N_TOK, EMB], BF16, kind="ExternalOutput")

    with tile.TileContext(nc) as tc:
        _emit(nc, tc, x_d, wq_d, wk_d, wv_d, wp_d, bqt_d, bkt_d, z_d)
    nc.finalize()
    return nc


def _emit(nc, tc, x_d, wq_d, wk_d, wv_d, wp_d, bqt_d, bkt_d, z_d):
    from contextlib import ExitStack

    ctx = ExitStack()
    with ctx:
        consts = ctx.enter_context(tc.tile_pool(name="consts", bufs=1))
        persist = ctx.enter_context(tc.tile_pool(name="persist", bufs=1))

        ident = consts.tile([P, P], BF16, tag="ident", name="ident")
        make_identity(nc, ident)
        eps_t = consts.tile([P, 1], F32, tag="eps", name="eps")
        nc.vector.memset(eps_t, 1e-5)
        nshift = consts.tile([P, 1], F32, tag="nshift", name="nshift")
        nc.vector.memset(nshift, -6.0)

        # weights: wq/wk as [128, 2(i), 8(h), 4(c), 64(m=d)]
        wq_s = persist.tile([P, 2, 8, 4, HD], FP8, tag="wq", name="wq")
        nc.sync.dma_start(out=wq_s, in_=wq_d[:, :].rearrange(
            "p (i h c m) -> p i h c m", i=2, h=8, c=4))
        wk_s = persist.tile([P, 2, 8, 4, HD], FP8, tag="wk", name="wk")
        nc.sync.dma_start(out=wk_s, in_=wk_d[:, :].rearrange(
            "p (i h c m) -> p i h c m", i=2, h=8, c=4))
        # wv as [128, 2(i), 4(c), 512(hd)]
        wv_s = persist.tile([P, 2, 4, 512], FP8, tag="wv", name="wv")
        nc.sync.dma_start(out=wv_s, in_=wv_d[:, :].rearrange(
            "p (i c m) -> p i c m", i=2, c=4))
        # wp as [128, 2(i), 2(g), 1024(n)]
        wp_s = persist.tile([P, 2, 2, EMB], FP8, tag="wp", name="wp")
        nc.sync.dma_start(out=wp_s, in_=wp_d[:, :].rearrange(
            "p (i g n) -> p i g n", i=2, g=2))
        bqt = consts.tile([HD, HPC], F32, tag="bqt", name="bqt")
        nc.sync.dma_start(out=bqt, in_=bqt_d[:, :])
        bkt = consts.tile([HD, HPC], F32, tag="bkt", name="bkt")
        nc.sync.dma_start(out=bkt, in_=bkt_d[:, :])

        ht = persist.tile([P, 8, N_TOK], FP8, tag="ht", name="ht")
        # per-head Q^T/K^T at base partition 0 (DoubleRow needs row pos 0);
        # full 128 partitions: d 0:64 at slice i=0, zero-pad elsewhere
        qt = persist.tile([P, HPC, 2, N_TOK], FP8, tag="qt", name="qt")
        kt = persist.tile([P, HPC, 2, N_TOK], FP8, tag="kt", name="kt")
        otn = [persist.tile([P, 2, N_TOK], FP8, tag=f"otn{g}", name=f"otn{g}")
               for g in range(2)]
        # cols 0:64 = V, cols 64:128 = 1/8: AV then leaves s/8 broadcast on
        # psum rows 64:128, so normalize is a plain 64-lane recip + mul
        vaug = [persist.tile([P, 2, HPC, P], FP8, tag=f"va{k}", name=f"va{k}")
                for k in range(NKT // 2)]

        # ---------------- Phase 1: LayerNorm + transpose ----------------
        with tc.tile_pool(name="ln", bufs=3) as ln_pool, \
             tc.tile_pool(name="lns", bufs=6) as lns, \
             tc.tile_pool(name="ps_a", bufs=2, space="PSUM") as ps_a:
            for t in range(NT):
                x_t = ln_pool.tile([P, EMB], F32, tag="x", name="x")
                nc.sync.dma_start(out=x_t, in_=x_d[t * P:(t + 1) * P, :])
                stats = lns.tile([P, 2, 6], F32, tag="stats", name="stats")
                nc.vector.bn_stats(out=stats[:, 0, :], in_=x_t[:, 0:512])
                nc.vector.bn_stats(out=stats[:, 1, :], in_=x_t[:, 512:1024])
                mv = lns.tile([P, 2], F32, tag="mv", name="mv")
                nc.vector.bn_aggr(out=mv, in_=stats)
                sd = lns.tile([P, 1], F32, tag="sd", name="sd")
                nc.scalar.activation(out=sd, in_=mv[:, 1:2], func=AF.Sqrt,
                                     bias=eps_t, scale=1.0)
                rstd = lns.tile([P, 1], F32, tag="rstd", name="rstd")
                nc.vector.reciprocal(out=rstd, in_=sd)
                nmean = lns.tile([P, 1], F32, tag="nmean", name="nmean")
                nc.vector.tensor_scalar_mul(nmean, mv[:, 0:1], -1.0)
                h_t = ln_pool.tile([P, EMB], BF16, tag="h", name="h")
                nc.gpsimd.tensor_scalar(
                    out=h_t, in0=x_t, scalar1=nmean, scalar2=rstd,
                    op0=mybir.AluOpType.add, op1=mybir.AluOpType.mult)
                pt = ps_a.tile([P, 8, P], BF16, tag="tr", name="tr")
                for j in range(8):
                    nc.tensor.transpose(pt[:, j, :], h_t[:, j * P:(j + 1) * P],
                                        ident)
                nc.vector.tensor_copy(out=ht[:, :, t * P:(t + 1) * P], in_=pt)

        # zero-pad regions (emitted after LN so Pool runs LN normalize first)
        nc.gpsimd.memset(qt[:, :, 1, :], 0.0)
        nc.gpsimd.memset(kt[:, :, 1, :], 0.0)
        nc.gpsimd.memset(qt[HD:P, :, 0, :], 0.0)
        nc.gpsimd.memset(kt[HD:P, :, 0, :], 0.0)
        for k in range(NKT // 2):
            nc.gpsimd.memset(vaug[k][:, :, :, HD:P], 1.0 / OSCALE)

        # ---------------- Phase 2: QKV (fp8 DoubleRow) ----------------
        with tc.tile_pool(name="ps_qkv", bufs=3, space="PSUM") as ps_qkv, \
             tc.tile_pool(name="ps_v", bufs=2, space="PSUM") as ps_v:
            for h in range(HPC):
                for half in range(2):
                    pq = ps_qkv.tile([HD, 2, 512], F32, tag="qkv", name="qkv")
                    for n2 in range(2):
                        n = 2 * half + n2
                        for c in range(4):
                            nc.tensor.matmul(
                                pq[:, n2, :], lhsT=wq_s[:, :, h, c, :],
                                rhs=ht[:, 2 * c:2 * c + 2, n * 512:(n + 1) * 512],
                                start=(c == 0), stop=(c == 3), perf_mode=DR)
                    nc.vector.tensor_scalar(
                        out=qt[0:HD, h, 0, half * 1024:(half + 1) * 1024]
                        .rearrange("p (n m) -> p n m", n=2), in0=pq,
                        scalar1=1.0 / WSCALE, scalar2=bqt[:, h:h + 1],
                        op0=mybir.AluOpType.mult, op1=mybir.AluOpType.add)
                    pk = ps_qkv.tile([HD, 2, 512], F32, tag="qkv", name="qkv")
                    for n2 in range(2):
                        n = 2 * half + n2
                        for c in range(4):
                            nc.tensor.matmul(
                                pk[:, n2, :], lhsT=wk_s[:, :, h, c, :],
                                rhs=ht[:, 2 * c:2 * c + 2, n * 512:(n + 1) * 512],
                                start=(c == 0), stop=(c == 3), perf_mode=DR)
                    nc.vector.tensor_scalar(
                        out=kt[0:HD, h, 0, half * 1024:(half + 1) * 1024]
                        .rearrange("p (n m) -> p n m", n=2), in0=pk,
                        scalar1=1.0 / WSCALE, scalar2=bkt[:, h:h + 1],
                        op0=mybir.AluOpType.mult, op1=mybir.AluOpType.add)
            for t in range(NT):
                pv = ps_v.tile([P, 512], F32, tag="v", name="v")
                for c in range(4):
                    nc.tensor.matmul(
                        pv, lhsT=ht[:, 2 * c:2 * c + 2, t * P:(t + 1) * P],
                        rhs=wv_s[:, :, c, :],
                        start=(c == 0), stop=(c == 3), perf_mode=DR)
                nc.vector.tensor_scalar_mul(
                    vaug[t // 2][:, t % 2, :, 0:HD],
                    pv.rearrange("p (h d) -> p h d", h=HPC), 1.0 / WSCALE)

        # ---------------- Phase 3+4: attention + projection ----------------
        with tc.tile_pool(name="expp", bufs=20) as expp, \
             tc.tile_pool(name="att_sm", bufs=6) as att_sm, \
             tc.tile_pool(name="zst", bufs=4) as zst, \
             tc.tile_pool(name="ps_st", bufs=2, space="PSUM") as ps_st, \
             tc.tile_pool(name="ps_ot", bufs=2, space="PSUM") as ps_ot, \
             tc.tile_pool(name="ps_z", bufs=2, space="PSUM") as ps_z:

            def emit_st(h, q):
                """Scores + exp for one (head, q-chunk); returns E tiles."""
                ets = []
                for c in range(NKT // 2):
                    pst = ps_st.tile([P, 2, 512], F32, tag="st", name="st")
                    for i in range(2):
                        k = 2 * c + i
                        nc.tensor.matmul(
                            pst[:, i, :],
                            lhsT=kt[:, h, :, k * P:(k + 1) * P],
                            rhs=qt[:, h, :, q * 512:(q + 1) * 512],
                            start=True, stop=True, perf_mode=DR)
                    e_t = expp.tile([P, 2, 512], FP8, tag="e", name="e")
                    # shift by -6 so exp fits fp8e4 range (cancels in softmax)
                    nc.scalar.activation(out=e_t, in_=pst, func=AF.Exp,
                                         bias=nshift)
                    ets.append(e_t)
                return ets

            def emit_av(h, q, ets):
                """att@v + normalize for one (head, q-chunk)."""
                g = h // 4
                pot = ps_ot.tile([P, 512], F32, tag="ot", name="ot")
                for c in range(NKT // 2):
                    nc.tensor.matmul(pot, lhsT=vaug[c][:, :, h, :],
                                     rhs=ets[c],
                                     start=(c == 0), stop=(c == NKT // 2 - 1),
                                     perf_mode=DR)
                # rows 64:128 hold s/8 (ones-block of 1/8 in vaug)
                rec_sb = att_sm.tile([HD, 512], BF16, tag="rec", name="rec")
                with nc.allow_low_precision(reason="softmax recip; bf16 ample"):
                    nc.vector.reciprocal(out=rec_sb, in_=pot[HD:P, :])
                nc.vector.tensor_mul(
                    otn[g][64 * (h % 2):64 * (h % 2) + 64, (h % 4) // 2,
                           q * 512:(q + 1) * 512],
                    pot[0:HD, :], rec_sb)

            def emit_proj(t):
                """projection for one token tile (128 tokens x 1024 emb)."""
                for ec in range(2):
                    pz = ps_z.tile([P, 512], F32, tag="z", name="z")
                    for g in range(2):
                        nc.tensor.matmul(
                            pz, lhsT=otn[g][:, :, t * P:(t + 1) * P],
                            rhs=wp_s[:, :, g, ec * 512:(ec + 1) * 512],
                            start=(g == 0), stop=(g == 1), perf_mode=DR)
                    z_t = zst.tile([P, 512], BF16, tag="z", name="z")
                    nc.vector.tensor_scalar_mul(z_t, pz, 1.0 / (WSCALE * OSCALE))
                    nc.sync.dma_start(
                        out=z_d[t * P:(t + 1) * P, ec * 512:(ec + 1) * 512],
                        in_=z_t)

            prev = None
            for q in range(QCH):
                for h in range(HPC):
                    ets = emit_st(h, q)
                    if prev is not None:
                        emit_av(*prev)
                    prev = (h, q, ets)
                if q > 0:
                    for t in range(4 * (q - 1), 4 * q):
                        emit_proj(t)
            emit_av(*prev)
            for t in range(4 * (QCH - 1), NT):
                emit_proj(t)


_CACHE = {}


def _get_nc():
    if "nc" not in _CACHE:
        _CACHE["nc"] = build_nc()
    return _CACHE["nc"]


def _prep_in_maps(x, ln_w, ln_b, w_qkv, b_qkv, w_proj, b_proj):
    bf = ml_dtypes.bfloat16
    f8 = ml_dtypes.float8_e4m3fn
    x = np.asarray(x, np.float32)
    ln_w = np.asarray(ln_w, np.float32)
    ln_b = np.asarray(ln_b, np.float32)
    w_qkv = np.asarray(w_qkv, np.float32)
    b_qkv = np.asarray(b_qkv, np.float32)
    w_proj = np.asarray(w_proj, np.float32)
    b_proj = np.asarray(b_proj, np.float32)

    b_eff = b_qkv + ln_b @ w_qkv
    w_eff = ln_w[:, None] * w_qkv
    w4 = w_eff.reshape(EMB, HEADS, HD, 3)
    b4 = b_eff.reshape(HEADS, HD, 3)
    wq = w4[..., 0] * SCALE
    wk = w4[..., 1]
    wv = w4[..., 2]
    bq = b4[..., 0] * SCALE
    bk = b4[..., 1]
    bv = b4[..., 2]

    def pack_qk(w, hsl):
        # w [EMB, 8 heads, 64] -> [128p, 2i, 8h, 4c, 64d] fp8 (x WSCALE)
        # emb = 256c + 128i + p
        wh = w[:, hsl, :]                                    # [1024, 8, 64]
        wh = wh.reshape(4, 2, P, HPC, HD)                    # c i p h d
        wh = wh.transpose(2, 1, 3, 0, 4)                     # p i h c d
        return np.ascontiguousarray((wh * WSCALE).reshape(P, -1)).astype(f8)

    def pack_qk_bias(b, hsl):
        # b [8 heads, 64] -> [64d, 8h] f32
        return np.ascontiguousarray(b[hsl].T.astype(np.float32))

    def pack_v(w, hsl):
        # w [EMB, 8, 64] -> [128p, 2i, 4c, 512hd] fp8 (x WSCALE)
        wh = w[:, hsl, :].reshape(4, 2, P, 512)              # c i p hd
        wh = wh.transpose(2, 1, 0, 3)                        # p i c hd
        return np.ascontiguousarray((wh * WSCALE).reshape(P, -1)).astype(f8)

    def pack_wp(w, hg):
        # w_proj rows for this head group [512, 1024] -> [128p, 2i, 2g, 1024]
        wh = w[hg * 512:(hg + 1) * 512, :]                   # hd=256g+128i+p
        wh = wh.reshape(2, 2, P, EMB).transpose(2, 1, 0, 3)  # p i g n
        return np.ascontiguousarray((wh * WSCALE).reshape(P, -1)).astype(f8)

    in_maps = []
    for cid in range(N_CORES):
        bi, hg = divmod(cid, 2)
        hsl = slice(hg * HPC, (hg + 1) * HPC)
        in_maps.append({
            "x": np.ascontiguousarray(x[bi]),
            "wq": pack_qk(wq, hsl),
            "wk": pack_qk(wk, hsl),
            "wv": pack_v(wv, hsl),
            "wp": pack_wp(w_proj, hg),
            "bqt": pack_qk_bias(bq, hsl),
            "bkt": pack_qk_bias(bk, hsl),
        })
    return in_maps


def _gather(results, x, b_proj_eff):
    x = np.asarray(x, np.float32)
    out = np.empty((x.shape[0], N_TOK, EMB), np.float32)
    for bi in range(x.shape[0]):
        out[bi] = (results[2 * bi]["z"].astype(np.float32)
                   + results[2 * bi + 1]["z"].astype(np.float32)
                   + b_proj_eff[None, :] + x[bi])
    return out


def _run(inputs, **kw):
    in_maps = _prep_in_maps(**inputs)
    # exact fold of V bias into projection bias
    b_eff = np.asarray(inputs["b_qkv"], np.float32) + \
        np.asarray(inputs["ln_b"], np.float32) @ np.asarray(
            inputs["w_qkv"], np.float32)
    bv = b_eff.reshape(HEADS, HD, 3)[..., 2].reshape(HEADS * HD)
    b_proj_eff = np.asarray(inputs["b_proj"], np.float32) + \
        bv @ np.asarray(inputs["w_proj"], np.float32)
    res = run_bass_kernel_spmd(_get_nc(), in_maps,
                               core_ids=list(range(N_CORES)), **kw)
    out = _gather(res.results, inputs["x"], b_proj_eff)
    return out, res


def kernel(**inputs):
    out, _ = _run(inputs)
    return out


# revision 47
# speedup vs baseline: 1.3902x; 1.0268x over previous
"""Trainium2 Bass kernel for a pre-LN multi-head attention block.

Full-input contract: kernel(**inputs) takes the unsharded tensors from
setup_inputs() and returns the full [4, 2048, 1024] output.

Sharding: 8 cores = 4 batches x 2 head-groups (8 heads each).
Each core computes LayerNorm(x[b]), its 8 heads of QKV + attention, and a
partial projection.  Host sums the two partials per batch and adds
b_proj_eff + residual.

Numerics strategy: all matmuls run in fp8e4 (e4m3) DoubleRow perf mode
(2 contraction sub-rows per instruction at 0.5 cycles/row).  Weights are
pre-scaled x32 into fp8's representable range host-side; the 1/32 is
folded into the PSUM->SBUF cast passes.  The attention path is only ~8%
of the output norm (residual dominates), so fp8's ~6% element error
lands at ~2e-3 total relative error, well under tolerance.

Host-side algebraic folds (exact):
  - ln_w folded into w_qkv columns, ln_b folded into b_qkv
  - softmax scale folded into W_q / b_q
  - V bias folded into b_proj (attention rows sum to 1)

Layouts (DoubleRow pairs contraction rows along a middle dim of 2):
  ht    [128, 8, 2048] fp8: ht[p, 2c+i, t] = h[t, emb=256c+128i+p]
  qt/kt [g][128, 2, 2048] fp8: [32s+j, r, t] = Q[t, head 4g+s, d=32r+j]
  vaug  [kk][128, 2, 8, 65] fp8: [p, i, h, d] = V[tok=256kk+128i+p, h, d],
        d=64 column = 1.0 (softmax-sum row trick)
  E     [128, 2, 512] fp8 per (k-pair, q-chunk): [p, i, q]=exp(S[k,q])
  otn   [g][128, 2, 2048] fp8: [p, i, t] = 8*OT_norm[hd=256g+128i+p, t]
"""

import sys

sys.path.insert(0, "/opt/trn_rl_repo")

import numpy as np
import ml_dtypes

import concourse.bass as bass
from concourse import bacc
import concourse.tile as tile
from concourse import mybir
from concourse.bass_utils import run_bass_kernel_spmd
from concourse.masks import make_identity

EMB = 1024
HEADS = 16
HD = 64
SCALE = HD ** -0.5
N_TOK = 2048
N_CORES = 8
HPC = 8                 # heads per core
QK_COLS = HPC * HD      # 512
P = 128
NT = N_TOK // P         # 16 token tiles
QCH = 4                 # q chunks of 512
NKT = 16                # k tiles of 128
WSCALE = 32.0           # host pre-scale on all weights (fp8 range)
OSCALE = 8.0            # extra scale on normalized attn output

BF16 = mybir.dt.bfloat16
F32 = mybir.dt.float32
FP8 = mybir.dt.float8e4
AF = mybir.ActivationFunctionType
DR = mybir.MatmulPerfMode.DoubleRow


def build_nc():
    nc = bacc.Bacc(trn_type="TRN2", target_bir_lowering=False)

    x_d = nc.dram_tensor("x", [N_TOK, EMB], F32, kind="ExternalInput")
    wq_d = nc.dram_tensor("wq", [P, 2 * 8 * 4 * HD], FP8, kind="ExternalInput")
    wk_d = nc.dram_tensor("wk", [P, 2 * 8 * 4 * HD], FP8, kind="ExternalInput")
    wv_d = nc.dram_tensor("wv", [P, 2 * 4 * 512], FP8, kind="ExternalInput")
    wp_d = nc.dram_tensor("wp", [P, 2 * 2 * EMB], FP8, kind="ExternalInput")
    bqt_d = nc.dram_tensor("bqt", [HD, HPC], F32, kind="ExternalInput")
    bkt_d = nc.dram_tensor("bkt", [HD, HPC], F32, kind="ExternalInput")
    z_d = nc.dram_tensor("z", [N_TOK, EMB], BF16, kind="ExternalOutput")

    with tile.TileContext(nc) as tc:
        _emit(nc, tc, x_d, wq_d, wk_d, wv_d, wp_d, bqt_d, bkt_d, z_d)
    nc.finalize()
    return nc


def _emit(nc, tc, x_d, wq_d, wk_d, wv_d, wp_d, bqt_d, bkt_d, z_d):
    from contextlib import ExitStack

    ctx = ExitStack()
    with ctx:
        consts = ctx.enter_context(tc.tile_pool(name="consts", bufs=1))
        persist = ctx.enter_context(tc.tile_pool(name="persist", bufs=1))

        eps_t = consts.tile([P, 1], F32, tag="eps", name="eps")
        nc.vector.memset(eps_t, 1e-5)
        nshift = consts.tile([P, 1], F32, tag="nshift", name="nshift")
        nc.vector.memset(nshift, -6.0)

        # weights: wq/wk as [128, 2(i), 8(h), 4(c), 64(m=d)]
        wq_s = persist.tile([P, 2, 8, 4, HD], FP8, tag="wq", name="wq")
        nc.sync.dma_start(out=wq_s, in_=wq_d[:, :].rearrange(
            "p (i h c m) -> p i h c m", i=2, h=8, c=4))
        wk_s = persist.tile([P, 2, 8, 4, HD], FP8, tag="wk", name="wk")
        nc.sync.dma_start(out=wk_s, in_=wk_d[:, :].rearrange(
            "p (i h c m) -> p i h c m", i=2, h=8, c=4))
        # wv as [128, 2(i), 4(c), 512(hd)]
        wv_s = persist.tile([P, 2, 4, 512], FP8, tag="wv", name="wv")
        nc.sync.dma_start(out=wv_s, in_=wv_d[:, :].rearrange(
            "p (i c m) -> p i c m", i=2, c=4))
        # wp as [128, 2(i), 2(g), 1024(n)]
        wp_s = persist.tile([P, 2, 2, EMB], FP8, tag="wp", name="wp")
        nc.sync.dma_start(out=wp_s, in_=wp_d[:, :].rearrange(
            "p (i g n) -> p i g n", i=2, g=2))
        bqt = consts.tile([HD, HPC], F32, tag="bqt", name="bqt")
        nc.sync.dma_start(out=bqt, in_=bqt_d[:, :])
        bkt = consts.tile([HD, HPC], F32, tag="bkt", name="bkt")
        nc.sync.dma_start(out=bkt, in_=bkt_d[:, :])

        ht = persist.tile([P, 8, N_TOK], FP8, tag="ht", name="ht")
        htb = persist.tile([P, 8, N_TOK], BF16, tag="htb", name="htb")
        # per-head Q^T/K^T at base partition 0 (DoubleRow needs row pos 0);
        # full 128 partitions: d 0:64 at slice i=0, zero-pad elsewhere
        qt = persist.tile([P, HPC, 2, N_TOK], FP8, tag="qt", name="qt")
        kt = persist.tile([P, HPC, 2, N_TOK], FP8, tag="kt", name="kt")
        otn = [persist.tile([P, 2, N_TOK], FP8, tag=f"otn{g}", name=f"otn{g}")
               for g in range(2)]
        # cols 0:64 = V, cols 64:128 = 1/8: AV then leaves s/8 broadcast on
        # psum rows 64:128, so normalize is a plain 64-lane recip + mul
        vaug = [persist.tile([P, 2, HPC, P], FP8, tag=f"va{k}", name=f"va{k}")
                for k in range(NKT // 2)]

        # ---------------- Phase 1: LayerNorm + transpose ----------------
        with tc.tile_pool(name="ln", bufs=3) as ln_pool, \
             tc.tile_pool(name="lns", bufs=6) as lns:
            for t in range(NT):
                x_t = ln_pool.tile([P, EMB], F32, tag="x", name="x")
                nc.sync.dma_start(out=x_t, in_=x_d[t * P:(t + 1) * P, :])
                stats = lns.tile([P, 2, 6], F32, tag="stats", name="stats")
                nc.vector.bn_stats(out=stats[:, 0, :], in_=x_t[:, 0:512])
                nc.vector.bn_stats(out=stats[:, 1, :], in_=x_t[:, 512:1024])
                mv = lns.tile([P, 2], F32, tag="mv", name="mv")
                nc.vector.bn_aggr(out=mv, in_=stats)
                sd = lns.tile([P, 1], F32, tag="sd", name="sd")
                nc.scalar.activation(out=sd, in_=mv[:, 1:2], func=AF.Sqrt,
                                     bias=eps_t, scale=1.0)
                rstd = lns.tile([P, 1], F32, tag="rstd", name="rstd")
                nc.vector.reciprocal(out=rstd, in_=sd)
                nmean = lns.tile([P, 1], F32, tag="nmean", name="nmean")
                nc.vector.tensor_scalar_mul(nmean, mv[:, 0:1], -1.0)
                h_t = ln_pool.tile([P, EMB], BF16, tag="h", name="h")
                nc.gpsimd.tensor_scalar(
                    out=h_t, in0=x_t, scalar1=nmean, scalar2=rstd,
                    op0=mybir.AluOpType.add, op1=mybir.AluOpType.mult)
                # transpose via DMA XBAR (2-byte only), split across the two
                # HWDGE queues (SP + ACT); then one DVE cast pass to fp8
                for j in range(8):
                    eng = nc.sync if j % 2 == 0 else nc.scalar
                    eng.dma_start(
                        out=htb[:, j, t * P:(t + 1) * P],
                        in_=h_t[:, j * P:(j + 1) * P], transpose=True)
                nc.vector.tensor_copy(out=ht[:, :, t * P:(t + 1) * P],
                                      in_=htb[:, :, t * P:(t + 1) * P])

        # zero-pad regions (emitted after LN so Pool runs LN normalize first)
        nc.gpsimd.memset(qt[:, :, 1, :], 0.0)
        nc.gpsimd.memset(kt[:, :, 1, :], 0.0)
        nc.gpsimd.memset(qt[HD:P, :, 0, :], 0.0)
        nc.gpsimd.memset(kt[HD:P, :, 0, :], 0.0)
        for k in range(NKT // 2):
            nc.gpsimd.memset(vaug[k][:, :, :, HD:P], 1.0 / OSCALE)

        # ---------------- Phase 2: QKV (fp8 DoubleRow) ----------------
        with tc.tile_pool(name="ps_qkv", bufs=3, space="PSUM") as ps_qkv, \
             tc.tile_pool(name="ps_v", bufs=2, space="PSUM") as ps_v:
            for h in range(HPC):
                for half in range(2):
                    pq = ps_qkv.tile([HD, 2, 512], F32, tag="qkv", name="qkv")
                    for n2 in range(2):
                        n = 2 * half + n2
                        for c in range(4):
                            nc.tensor.matmul(
                                pq[:, n2, :], lhsT=wq_s[:, :, h, c, :],
                                rhs=ht[:, 2 * c:2 * c + 2, n * 512:(n + 1) * 512],
                                start=(c == 0), stop=(c == 3), perf_mode=DR)
                    nc.vector.tensor_scalar(
                        out=qt[0:HD, h, 0, half * 1024:(half + 1) * 1024]
                        .rearrange("p (n m) -> p n m", n=2), in0=pq,
                        scalar1=1.0 / WSCALE, scalar2=bqt[:, h:h + 1],
                        op0=mybir.AluOpType.mult, op1=mybir.AluOpType.add)
                    pk = ps_qkv.tile([HD, 2, 512], F32, tag="qkv", name="qkv")
                    for n2 in range(2):
                        n = 2 * half + n2
                        for c in range(4):
                            nc.tensor.matmul(
                                pk[:, n2, :], lhsT=wk_s[:, :, h, c, :],
                                rhs=ht[:, 2 * c:2 * c + 2, n * 512:(n + 1) * 512],
                                start=(c == 0), stop=(c == 3), perf_mode=DR)
                    nc.vector.tensor_scalar(
                        out=kt[0:HD, h, 0, half * 1024:(half + 1) * 1024]
                        .rearrange("p (n m) -> p n m", n=2), in0=pk,
                        scalar1=1.0 / WSCALE, scalar2=bkt[:, h:h + 1],
                        op0=mybir.AluOpType.mult, op1=mybir.AluOpType.add)
            for t in range(NT):
                pv = ps_v.tile([P, 512], F32, tag="v", name="v")
                for c in range(4):
                    nc.tensor.matmul(
                        pv, lhsT=ht[:, 2 * c:2 * c + 2, t * P:(t + 1) * P],
                        rhs=wv_s[:, :, c, :],
                        start=(c == 0), stop=(c == 3), perf_mode=DR)
                nc.vector.tensor_scalar_mul(
                    vaug[t // 2][:, t % 2, :, 0:HD],
                    pv.rearrange("p (h d) -> p h d", h=HPC), 1.0 / WSCALE)

        # ---------------- Phase 3+4: attention + projection ----------------
        with tc.tile_pool(name="expp", bufs=16) as expp, \
             tc.tile_pool(name="att_sm", bufs=4) as att_sm, \
             tc.tile_pool(name="zst", bufs=4) as zst, \
             tc.tile_pool(name="ps_st", bufs=3, space="PSUM") as ps_st, \
             tc.tile_pool(name="ps_ot", bufs=2, space="PSUM") as ps_ot:

            def emit_st(h, q):
                """Scores + exp for one (head, q-chunk); returns E tiles."""
                ets = []
                for c in range(NKT // 2):
                    pst = ps_st.tile([P, 2, 512], F32, tag="st", name="st")
                    for i in range(2):
                        k = 2 * c + i
                        nc.tensor.matmul(
                            pst[:, i, :],
                            lhsT=kt[:, h, :, k * P:(k + 1) * P],
                            rhs=qt[:, h, :, q * 512:(q + 1) * 512],
                            start=True, stop=True, perf_mode=DR)
                    e_t = expp.tile([P, 2, 512], FP8, tag="e", name="e")
                    # shift by -6 so exp fits fp8e4 range (cancels in softmax)
                    nc.scalar.activation(out=e_t, in_=pst, func=AF.Exp,
                                         bias=nshift)
                    ets.append(e_t)
                return ets

            def emit_av(h, q, ets):
                """att@v + normalize for one (head, q-chunk)."""
                g = h // 4
                pot = ps_ot.tile([P, 512], F32, tag="ot", name="ot")
                for c in range(NKT // 2):
                    nc.tensor.matmul(pot, lhsT=vaug[c][:, :, h, :],
                                     rhs=ets[c],
                                     start=(c == 0), stop=(c == NKT // 2 - 1),
                                     perf_mode=DR)
                # rows 64:128 hold s/8 (ones-block of 1/8 in vaug)
                rec_sb = att_sm.tile([HD, 512], BF16, tag="rec", name="rec")
                with nc.allow_low_precision(reason="softmax recip; bf16 ample"):
                    nc.vector.reciprocal(out=rec_sb, in_=pot[HD:P, :])
                nc.vector.tensor_mul(
                    otn[g][64 * (h % 2):64 * (h % 2) + 64, (h % 4) // 2,
                           q * 512:(q + 1) * 512],
                    pot[0:HD, :], rec_sb)

            def emit_proj(t):
                """projection for one token tile (128 tokens x 1024 emb)."""
                for ec in range(2):
                    pz = ps_ot.tile([P, 512], F32, tag="ot", name="z")
                    for g in range(2):
                        nc.tensor.matmul(
                            pz, lhsT=otn[g][:, :, t * P:(t + 1) * P],
                            rhs=wp_s[:, :, g, ec * 512:(ec + 1) * 512],
                            start=(g == 0), stop=(g == 1), perf_mode=DR)
                    z_t = zst.tile([P, 512], BF16, tag="z", name="z")
                    nc.vector.tensor_scalar_mul(z_t, pz, 1.0 / (WSCALE * OSCALE))
                    nc.sync.dma_start(
                        out=z_d[t * P:(t + 1) * P, ec * 512:(ec + 1) * 512],
                        in_=z_t)

            prev = None
            for q in range(QCH):
                for h in range(HPC):
                    ets = emit_st(h, q)
                    if prev is not None:
                        emit_av(*prev)
                    prev = (h, q, ets)
                if q > 0:
                    for t in range(4 * (q - 1), 4 * q):
                        emit_proj(t)
            emit_av(*prev)
            for t in range(4 * (QCH - 1), NT):
                emit_proj(t)


_CACHE = {}


def _get_nc():
    if "nc" not in _CACHE:
        _CACHE["nc"] = build_nc()
    return _CACHE["nc"]


def _prep_in_maps(x, ln_w, ln_b, w_qkv, b_qkv, w_proj, b_proj):
    bf = ml_dtypes.bfloat16
    f8 = ml_dtypes.float8_e4m3fn
    x = np.asarray(x, np.float32)
    ln_w = np.asarray(ln_w, np.float32)
    ln_b = np.asarray(ln_b, np.float32)
    w_qkv = np.asarray(w_qkv, np.float32)
    b_qkv = np.asarray(b_qkv, np.float32)
    w_proj = np.asarray(w_proj, np.float32)
    b_proj = np.asarray(b_proj, np.float32)

    b_eff = b_qkv + ln_b @ w_qkv
    w_eff = ln_w[:, None] * w_qkv
    w4 = w_eff.reshape(EMB, HEADS, HD, 3)
    b4 = b_eff.reshape(HEADS, HD, 3)
    wq = w4[..., 0] * SCALE
    wk = w4[..., 1]
    wv = w4[..., 2]
    bq = b4[..., 0] * SCALE
    bk = b4[..., 1]
    bv = b4[..., 2]

    def pack_qk(w, hsl):
        # w [EMB, 8 heads, 64] -> [128p, 2i, 8h, 4c, 64d] fp8 (x WSCALE)
        # emb = 256c + 128i + p
        wh = w[:, hsl, :]                                    # [1024, 8, 64]
        wh = wh.reshape(4, 2, P, HPC, HD)                    # c i p h d
        wh = wh.transpose(2, 1, 3, 0, 4)                     # p i h c d
        return np.ascontiguousarray((wh * WSCALE).reshape(P, -1)).astype(f8)

    def pack_qk_bias(b, hsl):
        # b [8 heads, 64] -> [64d, 8h] f32
        return np.ascontiguousarray(b[hsl].T.astype(np.float32))

    def pack_v(w, hsl):
        # w [EMB, 8, 64] -> [128p, 2i, 4c, 512hd] fp8 (x WSCALE)
        wh = w[:, hsl, :].reshape(4, 2, P, 512)              # c i p hd
        wh = wh.transpose(2, 1, 0, 3)                        # p i c hd
        return np.ascontiguousarray((wh * WSCALE).reshape(P, -1)).astype(f8)

    def pack_wp(w, hg):
        # w_proj rows for this head group [512, 1024] -> [128p, 2i, 2g, 1024]
        wh = w[hg * 512:(hg + 1) * 512, :]                   # hd=256g+128i+p
        wh = wh.reshape(2, 2, P, EMB).transpose(2, 1, 0, 3)  # p i g n
        return np.ascontiguousarray((wh * WSCALE).reshape(P, -1)).astype(f8)

    in_maps = []
    for cid in range(N_CORES):
        bi, hg = divmod(cid, 2)
        hsl = slice(hg * HPC, (hg + 1) * HPC)
        in_maps.append({
            "x": np.ascontiguousarray(x[bi]),
            "wq": pack_qk(wq, hsl),
            "wk": pack_qk(wk, hsl),
            "wv": pack_v(wv, hsl),
            "wp": pack_wp(w_proj, hg),
            "bqt": pack_qk_bias(bq, hsl),
            "bkt": pack_qk_bias(bk, hsl),
        })
    return in_maps


def _gather(results, x, b_proj_eff):
    x = np.asarray(x, np.float32)
    out = np.empty((x.shape[0], N_TOK, EMB), np.float32)
    for bi in range(x.shape[0]):
        out[bi] = (results[2 * bi]["z"].astype(np.float32)
                   + results[2 * bi + 1]["z"].astype(np.float32)
                   + b_proj_eff[None, :] + x[bi])
    return out


def _run(inputs, **kw):
    in_maps = _prep_in_maps(**inputs)
    # exact fold of V bias into projection bias
    b_eff = np.asarray(inputs["b_qkv"], np.float32) + \
        np.asarray(inputs["ln_b"], np.float32) @ np.asarray(
            inputs["w_qkv"], np.float32)
    bv = b_eff.reshape(HEADS, HD, 3)[..., 2].reshape(HEADS * HD)
    b_proj_eff = np.asarray(inputs["b_proj"], np.float32) + \
        bv @ np.asarray(inputs["w_proj"], np.float32)
    res = run_bass_kernel_spmd(_get_nc(), in_maps,
                               core_ids=list(range(N_CORES)), **kw)
    out = _gather(res.results, inputs["x"], b_proj_eff)
    return out, res


def kernel(**inputs):
    out, _ = _run(inputs)
    return out


# revision 54
# speedup vs baseline: 1.5961x; 1.1481x over previous
"""Trainium2 Bass kernel for a pre-LN multi-head attention block.

Full-input contract: kernel(**inputs) takes the unsharded tensors from
setup_inputs() and returns the full [4, 2048, 1024] output.

Sharding: 8 cores = 4 batches x 2 head-groups (8 heads each).
Each core computes LayerNorm(x[b]), its 8 heads of QKV + attention, and a
partial projection.  Host sums the two partials per batch and adds
b_proj_eff + residual.

Numerics strategy: all matmuls run in fp8e4 (e4m3) DoubleRow perf mode
(2 contraction sub-rows per instruction at 0.5 cycles/row).  Weights are
pre-scaled x32 into fp8's representable range host-side; the 1/32 is
folded into the PSUM->SBUF cast passes.  The attention path is only ~8%
of the output norm (residual dominates), so fp8's ~6% element error
lands at ~2e-3 total relative error, well under tolerance.

Host-side algebraic folds (exact):
  - ln_w folded into w_qkv columns, ln_b folded into b_qkv
  - softmax scale folded into W_q / b_q
  - V bias folded into b_proj (attention rows sum to 1)

Layouts (DoubleRow pairs contraction rows along a middle dim of 2):
  ht    [128, 8, 2048] fp8: ht[p, 2c+i, t] = h[t, emb=256c+128i+p]
  qt/kt [g][128, 2, 2048] fp8: [32s+j, r, t] = Q[t, head 4g+s, d=32r+j]
  vaug  [kk][128, 2, 8, 65] fp8: [p, i, h, d] = V[tok=256kk+128i+p, h, d],
        d=64 column = 1.0 (softmax-sum row trick)
  E     [128, 2, 512] fp8 per (k-pair, q-chunk): [p, i, q]=exp(S[k,q])
  otn   [g][128, 2, 2048] fp8: [p, i, t] = 8*OT_norm[hd=256g+128i+p, t]
"""

import sys

sys.path.insert(0, "/opt/trn_rl_repo")

import numpy as np
import ml_dtypes

import concourse.bass as bass
from concourse import bacc
import concourse.tile as tile
from concourse import mybir
from concourse.bass_utils import run_bass_kernel_spmd
from concourse.masks import make_identity

EMB = 1024
HEADS = 16
HD = 64
SCALE = HD ** -0.5
N_TOK = 2048
N_CORES = 8
HPC = 8                 # heads per core
QK_COLS = HPC * HD      # 512
P = 128
NT = N_TOK // P         # 16 token tiles
QCH = 4                 # q chunks of 512
NKT = 16                # k tiles of 128
WSCALE = 32.0           # host pre-scale on all weights (fp8 range)
OSCALE = 8.0            # extra scale on normalized attn output

BF16 = mybir.dt.bfloat16
F32 = mybir.dt.float32
FP8 = mybir.dt.float8e4
AF = mybir.ActivationFunctionType
DR = mybir.MatmulPerfMode.DoubleRow


def build_nc():
    nc = bacc.Bacc(trn_type="TRN2", target_bir_lowering=False)

    x_d = nc.dram_tensor("x", [N_TOK, EMB], F32, kind="ExternalInput")
    wq_d = nc.dram_tensor("wq", [P, 2 * 8 * 4 * HD], FP8, kind="ExternalInput")
    wk_d = nc.dram_tensor("wk", [P, 2 * 8 * 4 * HD], FP8, kind="ExternalInput")
    wv_d = nc.dram_tensor("wv", [P, 2 * 4 * 512], FP8, kind="ExternalInput")
    wp_d = nc.dram_tensor("wp", [P, 2 * 2 * EMB], FP8, kind="ExternalInput")
    bqt_d = nc.dram_tensor("bqt", [HD, HPC], F32, kind="ExternalInput")
    bkt_d = nc.dram_tensor("bkt", [HD, HPC], F32, kind="ExternalInput")
    zpad_d = nc.dram_tensor("zpad", [P, 8 * N_TOK], FP8, kind="ExternalInput")
    vcst_d = nc.dram_tensor("vcst", [P, 2 * 8 * HD], FP8, kind="ExternalInput")
    z_d = nc.dram_tensor("z", [N_TOK, EMB], BF16, kind="ExternalOutput")

    with tile.TileContext(nc) as tc:
        _emit(nc, tc, x_d, wq_d, wk_d, wv_d, wp_d, bqt_d, bkt_d,
              zpad_d, vcst_d, z_d)
    nc.finalize()
    return nc


def _emit(nc, tc, x_d, wq_d, wk_d, wv_d, wp_d, bqt_d, bkt_d,
          zpad_d, vcst_d, z_d):
    from contextlib import ExitStack

    ctx = ExitStack()
    with ctx:
        consts = ctx.enter_context(tc.tile_pool(name="consts", bufs=1))
        persist = ctx.enter_context(tc.tile_pool(name="persist", bufs=1))

        eps_t = consts.tile([P, 1], F32, tag="eps", name="eps")
        nc.vector.memset(eps_t, 1e-5)
        nshift = consts.tile([P, 1], F32, tag="nshift", name="nshift")
        nc.vector.memset(nshift, -6.0)

        # weights: wq/wk as [128, 2(i), 8(h), 4(c), 64(m=d)]
        wq_s = persist.tile([P, 2, 8, 4, HD], FP8, tag="wq", name="wq")
        nc.sync.dma_start(out=wq_s, in_=wq_d[:, :].rearrange(
            "p (i h c m) -> p i h c m", i=2, h=8, c=4))
        wk_s = persist.tile([P, 2, 8, 4, HD], FP8, tag="wk", name="wk")
        nc.sync.dma_start(out=wk_s, in_=wk_d[:, :].rearrange(
            "p (i h c m) -> p i h c m", i=2, h=8, c=4))
        # wv as [128, 2(i), 4(c), 512(hd)]
        wv_s = persist.tile([P, 2, 4, 512], FP8, tag="wv", name="wv")
        nc.sync.dma_start(out=wv_s, in_=wv_d[:, :].rearrange(
            "p (i c m) -> p i c m", i=2, c=4))
        # wp as [128, 2(i), 2(g), 1024(n)]
        wp_s = persist.tile([P, 2, 2, EMB], FP8, tag="wp", name="wp")
        nc.sync.dma_start(out=wp_s, in_=wp_d[:, :].rearrange(
            "p (i g n) -> p i g n", i=2, g=2))
        bqt = consts.tile([HD, HPC], F32, tag="bqt", name="bqt")
        nc.sync.dma_start(out=bqt, in_=bqt_d[:, :])
        bkt = consts.tile([HD, HPC], F32, tag="bkt", name="bkt")
        nc.sync.dma_start(out=bkt, in_=bkt_d[:, :])

        ident = consts.tile([P, P], BF16, tag="ident", name="ident")
        make_identity(nc, ident)

        ht = persist.tile([P, 8, N_TOK], FP8, tag="ht", name="ht")
        # per-head Q^T/K^T at base partition 0 (DoubleRow needs row pos 0);
        # full 128 partitions: d 0:64 at slice i=0, zero-pad elsewhere
        qt = persist.tile([P, HPC, 2, N_TOK], FP8, tag="qt", name="qt")
        kt = persist.tile([P, HPC, 2, N_TOK], FP8, tag="kt", name="kt")
        otn = [persist.tile([P, 2, N_TOK], FP8, tag=f"otn{g}", name=f"otn{g}")
               for g in range(2)]
        # cols 0:64 = V, cols 64:128 = 1/8: AV then leaves s/8 broadcast on
        # psum rows 64:128, so normalize is a plain 64-lane recip + mul
        vaug = [persist.tile([P, 2, HPC, P], FP8, tag=f"va{k}", name=f"va{k}")
                for k in range(NKT // 2)]
        # zero-pad / const fills via DMA (keeps Pool free for LN normalize)
        nc.sync.dma_start(out=qt[:, :, 1, :], in_=zpad_d[:, :].rearrange(
            "p (h n) -> p h n", h=HPC))
        nc.sync.dma_start(out=kt[:, :, 1, :], in_=zpad_d[:, :].rearrange(
            "p (h n) -> p h n", h=HPC))
        nc.sync.dma_start(out=qt[HD:P, :, 0, :],
                          in_=zpad_d[HD:P, :].rearrange(
                              "p (h n) -> p h n", h=HPC))
        nc.sync.dma_start(out=kt[HD:P, :, 0, :],
                          in_=zpad_d[HD:P, :].rearrange(
                              "p (h n) -> p h n", h=HPC))
        for k in range(NKT // 2):
            nc.sync.dma_start(out=vaug[k][:, :, :, HD:P],
                              in_=vcst_d[:, :].rearrange(
                                  "p (i h d) -> p i h d", i=2, h=HPC))

        # ---- single shared PSUM pool for transposes/V/QK (via views) + ST --
        with tc.tile_pool(name="ln", bufs=5) as ln_pool, \
             tc.tile_pool(name="lns", bufs=3) as lns, \
             tc.tile_pool(name="expp", bufs=16) as expp, \
             tc.tile_pool(name="att_sm", bufs=4) as att_sm, \
             tc.tile_pool(name="zst", bufs=4) as zst, \
             tc.tile_pool(name="ps_st", bufs=3, space="PSUM") as ps_st, \
             tc.tile_pool(name="ps_ot", bufs=2, space="PSUM") as ps_ot:

            def ps_tile():
                return ps_st.tile([P, 2, 512], F32, tag="st", name="st")

            # ------------- Phase 1: LayerNorm + transpose + V -------------
            for nch in range(4):
                sd4 = lns.tile([P, 4], F32, tag="sd4", name="sd4")
                rstd4 = lns.tile([P, 4], F32, tag="rstd4", name="rstd4")
                nm4 = lns.tile([P, 4], F32, tag="nm4", name="nm4")
                mv4 = lns.tile([P, 4, 2], F32, tag="mv4", name="mv4")
                xts = []
                for t4 in range(4):
                    t = 4 * nch + t4
                    x_t = ln_pool.tile([P, EMB], F32, tag="x", name="x")
                    nc.sync.dma_start(out=x_t, in_=x_d[t * P:(t + 1) * P, :])
                    xts.append(x_t)
                    stats = lns.tile([P, 2, 6], F32, tag="stats", name="stats")
                    nc.vector.bn_stats(out=stats[:, 0, :], in_=x_t[:, 0:512])
                    nc.vector.bn_stats(out=stats[:, 1, :], in_=x_t[:, 512:1024])
                    nc.vector.bn_aggr(out=mv4[:, t4, :], in_=stats)
                    nc.scalar.activation(out=sd4[:, t4:t4 + 1],
                                         in_=mv4[:, t4, 1:2], func=AF.Sqrt,
                                         bias=eps_t, scale=1.0)
                nc.vector.reciprocal(out=rstd4, in_=sd4)
                nc.vector.tensor_scalar_mul(nm4, mv4[:, :, 0], -1.0)
                for t4 in range(4):
                    t = 4 * nch + t4
                    h_t = ln_pool.tile([P, EMB], BF16, tag="h", name="h")
                    nc.gpsimd.tensor_scalar(
                        out=h_t, in0=xts[t4],
                        scalar1=nm4[:, t4:t4 + 1], scalar2=rstd4[:, t4:t4 + 1],
                        op0=mybir.AluOpType.add, op1=mybir.AluOpType.mult)
                    pt = ps_tile().bitcast(BF16)[:, 0, :].rearrange(
                        "p (j m) -> p j m", j=8)
                    for j in range(8):
                        nc.tensor.transpose(pt[:, j, :],
                                            h_t[:, j * P:(j + 1) * P], ident)
                    nc.scalar.copy(out=ht[:, :, t * P:(t + 1) * P], in_=pt)
                    pv = ps_tile()[:, 0, :]
                    for c in range(4):
                        nc.tensor.matmul(
                            pv, lhsT=ht[:, 2 * c:2 * c + 2, t * P:(t + 1) * P],
                            rhs=wv_s[:, :, c, :],
                            start=(c == 0), stop=(c == 3), perf_mode=DR)
                    nc.vector.tensor_scalar_mul(
                        vaug[t // 2][:, t % 2, :, 0:HD],
                        pv.rearrange("p (h d) -> p h d", h=HPC), 1.0 / WSCALE)

            # ------------- QK for one head (interleaved below) -------------
            def emit_qk(h):
                for src_w, bias, dst in ((wq_s, bqt, qt), (wk_s, bkt, kt)):
                    for half in range(2):
                        pq = ps_tile()[0:HD, :, :]
                        for n2 in range(2):
                            n = 2 * half + n2
                            for c in range(4):
                                nc.tensor.matmul(
                                    pq[:, n2, :], lhsT=src_w[:, :, h, c, :],
                                    rhs=ht[:, 2 * c:2 * c + 2,
                                           n * 512:(n + 1) * 512],
                                    start=(c == 0), stop=(c == 3),
                                    perf_mode=DR)
                        nc.vector.tensor_scalar(
                            out=dst[0:HD, h, 0, half * 1024:(half + 1) * 1024]
                            .rearrange("p (n m) -> p n m", n=2), in0=pq,
                            scalar1=1.0 / WSCALE, scalar2=bias[:, h:h + 1],
                            op0=mybir.AluOpType.mult, op1=mybir.AluOpType.add)

            def emit_st(h, q):
                """Scores + exp for one (head, q-chunk); returns E tiles."""
                ets = []
                for c in range(NKT // 2):
                    pst = ps_tile()
                    for i in range(2):
                        k = 2 * c + i
                        nc.tensor.matmul(
                            pst[:, i, :],
                            lhsT=kt[:, h, :, k * P:(k + 1) * P],
                            rhs=qt[:, h, :, q * 512:(q + 1) * 512],
                            start=True, stop=True, perf_mode=DR)
                    e_t = expp.tile([P, 2, 512], FP8, tag="e", name="e")
                    # shift by -6 so exp fits fp8e4 range (cancels in softmax)
                    nc.scalar.activation(out=e_t, in_=pst, func=AF.Exp,
                                         bias=nshift)
                    ets.append(e_t)
                return ets

            def emit_av(h, q, ets):
                """att@v + normalize for one (head, q-chunk)."""
                g = h // 4
                pot = ps_ot.tile([P, 512], F32, tag="ot", name="ot")
                for c in range(NKT // 2):
                    nc.tensor.matmul(pot, lhsT=vaug[c][:, :, h, :],
                                     rhs=ets[c],
                                     start=(c == 0), stop=(c == NKT // 2 - 1),
                                     perf_mode=DR)
                # rows 64:128 hold s/8 (ones-block of 1/8 in vaug)
                rec_sb = att_sm.tile([HD, 512], BF16, tag="rec", name="rec")
                with nc.allow_low_precision(reason="softmax recip; bf16 ample"):
                    nc.vector.reciprocal(out=rec_sb, in_=pot[HD:P, :])
                nc.vector.tensor_mul(
                    otn[g][64 * (h % 2):64 * (h % 2) + 64, (h % 4) // 2,
                           q * 512:(q + 1) * 512],
                    pot[0:HD, :], rec_sb)

            def emit_proj(t):
                """projection for one token tile (128 tokens x 1024 emb)."""
                for ec in range(2):
                    pz = ps_ot.tile([P, 512], F32, tag="ot", name="z")
                    for g in range(2):
                        nc.tensor.matmul(
                            pz, lhsT=otn[g][:, :, t * P:(t + 1) * P],
                            rhs=wp_s[:, :, g, ec * 512:(ec + 1) * 512],
                            start=(g == 0), stop=(g == 1), perf_mode=DR)
                    z_t = zst.tile([P, 512], BF16, tag="z", name="z")
                    nc.vector.tensor_scalar_mul(z_t, pz, 1.0 / (WSCALE * OSCALE))
                    nc.sync.dma_start(
                        out=z_d[t * P:(t + 1) * P, ec * 512:(ec + 1) * 512],
                        in_=z_t)

            # attention h-outer; QK of head h+1 woven into head h's stream;
            # projection woven into the last head's stream
            emit_qk(0)
            prev = None
            for h in range(HPC):
                for q in range(QCH):
                    ets = emit_st(h, q)
                    if prev is not None:
                        emit_av(*prev)
                        ph, pq_, _ = prev
                        if ph == HPC - 1 and pq_ > 0:
                            for t in range(4 * (pq_ - 1), 4 * pq_):
                                emit_proj(t)
                    prev = (h, q, ets)
                    if q == 0 and h + 1 < HPC:
                        emit_qk(h + 1)
            emit_av(*prev)
            for t in range(4 * (QCH - 2), NT):
                emit_proj(t)


_CACHE = {}


def _get_nc():
    if "nc" not in _CACHE:
        _CACHE["nc"] = build_nc()
    return _CACHE["nc"]


def _prep_in_maps(x, ln_w, ln_b, w_qkv, b_qkv, w_proj, b_proj):
    bf = ml_dtypes.bfloat16
    f8 = ml_dtypes.float8_e4m3fn
    x = np.asarray(x, np.float32)
    ln_w = np.asarray(ln_w, np.float32)
    ln_b = np.asarray(ln_b, np.float32)
    w_qkv = np.asarray(w_qkv, np.float32)
    b_qkv = np.asarray(b_qkv, np.float32)
    w_proj = np.asarray(w_proj, np.float32)
    b_proj = np.asarray(b_proj, np.float32)

    b_eff = b_qkv + ln_b @ w_qkv
    w_eff = ln_w[:, None] * w_qkv
    w4 = w_eff.reshape(EMB, HEADS, HD, 3)
    b4 = b_eff.reshape(HEADS, HD, 3)
    wq = w4[..., 0] * SCALE
    wk = w4[..., 1]
    wv = w4[..., 2]
    bq = b4[..., 0] * SCALE
    bk = b4[..., 1]
    bv = b4[..., 2]

    def pack_qk(w, hsl):
        # w [EMB, 8 heads, 64] -> [128p, 2i, 8h, 4c, 64d] fp8 (x WSCALE)
        # emb = 256c + 128i + p
        wh = w[:, hsl, :]                                    # [1024, 8, 64]
        wh = wh.reshape(4, 2, P, HPC, HD)                    # c i p h d
        wh = wh.transpose(2, 1, 3, 0, 4)                     # p i h c d
        return np.ascontiguousarray((wh * WSCALE).reshape(P, -1)).astype(f8)

    def pack_qk_bias(b, hsl):
        # b [8 heads, 64] -> [64d, 8h] f32
        return np.ascontiguousarray(b[hsl].T.astype(np.float32))

    def pack_v(w, hsl):
        # w [EMB, 8, 64] -> [128p, 2i, 4c, 512hd] fp8 (x WSCALE)
        wh = w[:, hsl, :].reshape(4, 2, P, 512)              # c i p hd
        wh = wh.transpose(2, 1, 0, 3)                        # p i c hd
        return np.ascontiguousarray((wh * WSCALE).reshape(P, -1)).astype(f8)

    def pack_wp(w, hg):
        # w_proj rows for this head group [512, 1024] -> [128p, 2i, 2g, 1024]
        wh = w[hg * 512:(hg + 1) * 512, :]                   # hd=256g+128i+p
        wh = wh.reshape(2, 2, P, EMB).transpose(2, 1, 0, 3)  # p i g n
        return np.ascontiguousarray((wh * WSCALE).reshape(P, -1)).astype(f8)

    f8z = np.zeros((P, 8 * N_TOK), f8)
    vc = np.full((P, 2 * 8 * HD), 1.0 / OSCALE, f8)
    in_maps = []
    for cid in range(N_CORES):
        bi, hg = divmod(cid, 2)
        hsl = slice(hg * HPC, (hg + 1) * HPC)
        in_maps.append({
            "x": np.ascontiguousarray(x[bi]),
            "wq": pack_qk(wq, hsl),
            "wk": pack_qk(wk, hsl),
            "wv": pack_v(wv, hsl),
            "wp": pack_wp(w_proj, hg),
            "bqt": pack_qk_bias(bq, hsl),
            "bkt": pack_qk_bias(bk, hsl),
            "zpad": f8z,
            "vcst": vc,
        })
    return in_maps


def _gather(results, x, b_proj_eff):
    x = np.asarray(x, np.float32)
    out = np.empty((x.shape[0], N_TOK, EMB), np.float32)
    for bi in range(x.shape[0]):
        out[bi] = (results[2 * bi]["z"].astype(np.float32)
                   + results[2 * bi + 1]["z"].astype(np.float32)
                   + b_proj_eff[None, :] + x[bi])
    return out


def _run(inputs, **kw):
    in_maps = _prep_in_maps(**inputs)
    # exact fold of V bias into projection bias
    b_eff = np.asarray(inputs["b_qkv"], np.float32) + \
        np.asarray(inputs["ln_b"], np.float32) @ np.asarray(
            inputs["w_qkv"], np.float32)
    bv = b_eff.reshape(HEADS, HD, 3)[..., 2].reshape(HEADS * HD)
    b_proj_eff = np.asarray(inputs["b_proj"], np.float32) + \
        bv @ np.asarray(inputs["w_proj"], np.float32)
    res = run_bass_kernel_spmd(_get_nc(), in_maps,
                               core_ids=list(range(N_CORES)), **kw)
    out = _gather(res.results, inputs["x"], b_proj_eff)
    return out, res


def kernel(**inputs):
    out, _ = _run(inputs)
    return out


# revision 57
# speedup vs baseline: 1.7171x; 1.0758x over previous
"""Trainium2 Bass kernel for a pre-LN multi-head attention block.

Full-input contract: kernel(**inputs) takes the unsharded tensors from
setup_inputs() and returns the full [4, 2048, 1024] output.

Sharding: 8 cores = 4 batches x 2 head-groups (8 heads each).
Each core computes LayerNorm(x[b]), its 8 heads of QKV + attention, and a
partial projection.  Host sums the two partials per batch and adds
b_proj_eff + residual.

Numerics strategy: all matmuls run in fp8e4 (e4m3) DoubleRow perf mode
(2 contraction sub-rows per instruction at 0.5 cycles/row).  Weights are
pre-scaled x32 into fp8's representable range host-side; the 1/32 is
folded into the PSUM->SBUF cast passes.  The attention path is only ~8%
of the output norm (residual dominates), so fp8's ~6% element error
lands at ~2e-3 total relative error, well under tolerance.

Host-side algebraic folds (exact):
  - ln_w folded into w_qkv columns, ln_b folded into b_qkv
  - softmax scale folded into W_q / b_q
  - V bias folded into b_proj (attention rows sum to 1)

Layouts (DoubleRow pairs contraction rows along a middle dim of 2):
  ht    [128, 8, 2048] fp8: ht[p, 2c+i, t] = h[t, emb=256c+128i+p]
  qt/kt [g][128, 2, 2048] fp8: [32s+j, r, t] = Q[t, head 4g+s, d=32r+j]
  vaug  [kk][128, 2, 8, 65] fp8: [p, i, h, d] = V[tok=256kk+128i+p, h, d],
        d=64 column = 1.0 (softmax-sum row trick)
  E     [128, 2, 512] fp8 per (k-pair, q-chunk): [p, i, q]=exp(S[k,q])
  otn   [g][128, 2, 2048] fp8: [p, i, t] = 8*OT_norm[hd=256g+128i+p, t]
"""

import sys

sys.path.insert(0, "/opt/trn_rl_repo")

import numpy as np
import ml_dtypes

import concourse.bass as bass
from concourse import bacc
import concourse.tile as tile
from concourse import mybir
from concourse.bass_utils import run_bass_kernel_spmd
from concourse.masks import make_identity

EMB = 1024
HEADS = 16
HD = 64
SCALE = HD ** -0.5
N_TOK = 2048
N_CORES = 8
HPC = 8                 # heads per core
QK_COLS = HPC * HD      # 512
P = 128
NT = N_TOK // P         # 16 token tiles
QCH = 4                 # q chunks of 512
NKT = 16                # k tiles of 128
WSCALE = 32.0           # host pre-scale on all weights (fp8 range)
OSCALE = 8.0            # extra scale on normalized attn output

BF16 = mybir.dt.bfloat16
F32 = mybir.dt.float32
FP8 = mybir.dt.float8e4
AF = mybir.ActivationFunctionType
DR = mybir.MatmulPerfMode.DoubleRow


def build_nc():
    nc = bacc.Bacc(trn_type="TRN2", target_bir_lowering=False)

    x_d = nc.dram_tensor("x", [N_TOK, EMB], F32, kind="ExternalInput")
    wq_d = nc.dram_tensor("wq", [P, 2 * 8 * 4 * HD], FP8, kind="ExternalInput")
    wk_d = nc.dram_tensor("wk", [P, 2 * 8 * 4 * HD], FP8, kind="ExternalInput")
    wv_d = nc.dram_tensor("wv", [P, 2 * 4 * 512], FP8, kind="ExternalInput")
    wp_d = nc.dram_tensor("wp", [P, 2 * 2 * EMB], FP8, kind="ExternalInput")
    bqt_d = nc.dram_tensor("bqt", [HD, HPC], F32, kind="ExternalInput")
    bkt_d = nc.dram_tensor("bkt", [HD, HPC], F32, kind="ExternalInput")
    zpad_d = nc.dram_tensor("zpad", [P, 8 * N_TOK], FP8, kind="ExternalInput")
    vcst_d = nc.dram_tensor("vcst", [P, 2 * 8 * HD], FP8, kind="ExternalInput")
    z_d = nc.dram_tensor("z", [N_TOK, EMB], BF16, kind="ExternalOutput")

    with tile.TileContext(nc) as tc:
        _emit(nc, tc, x_d, wq_d, wk_d, wv_d, wp_d, bqt_d, bkt_d,
              zpad_d, vcst_d, z_d)
    nc.finalize()
    return nc


def _emit(nc, tc, x_d, wq_d, wk_d, wv_d, wp_d, bqt_d, bkt_d,
          zpad_d, vcst_d, z_d):
    from contextlib import ExitStack

    ctx = ExitStack()
    with ctx:
        consts = ctx.enter_context(tc.tile_pool(name="consts", bufs=1))
        persist = ctx.enter_context(tc.tile_pool(name="persist", bufs=1))

        eps_t = consts.tile([P, 1], F32, tag="eps", name="eps")
        nc.vector.memset(eps_t, 1e-5)
        nshift = consts.tile([P, 1], F32, tag="nshift", name="nshift")
        nc.vector.memset(nshift, -6.0)

        # weights: wq/wk as [128, 2(i), 8(h), 4(c), 64(m=d)]
        wq_s = persist.tile([P, 2, 8, 4, HD], FP8, tag="wq", name="wq")
        nc.sync.dma_start(out=wq_s, in_=wq_d[:, :].rearrange(
            "p (i h c m) -> p i h c m", i=2, h=8, c=4))
        wk_s = persist.tile([P, 2, 8, 4, HD], FP8, tag="wk", name="wk")
        nc.sync.dma_start(out=wk_s, in_=wk_d[:, :].rearrange(
            "p (i h c m) -> p i h c m", i=2, h=8, c=4))
        # wv as [128, 2(i), 4(c), 512(hd)]
        wv_s = persist.tile([P, 2, 4, 512], FP8, tag="wv", name="wv")
        nc.sync.dma_start(out=wv_s, in_=wv_d[:, :].rearrange(
            "p (i c m) -> p i c m", i=2, c=4))
        # wp as [128, 2(i), 2(g), 1024(n)]
        wp_s = persist.tile([P, 2, 2, EMB], FP8, tag="wp", name="wp")
        nc.sync.dma_start(out=wp_s, in_=wp_d[:, :].rearrange(
            "p (i g n) -> p i g n", i=2, g=2))
        bqt = consts.tile([HD, HPC], F32, tag="bqt", name="bqt")
        nc.sync.dma_start(out=bqt, in_=bqt_d[:, :])
        bkt = consts.tile([HD, HPC], F32, tag="bkt", name="bkt")
        nc.sync.dma_start(out=bkt, in_=bkt_d[:, :])

        ident = consts.tile([P, P], BF16, tag="ident", name="ident")
        make_identity(nc, ident)

        ht = persist.tile([P, 8, N_TOK], FP8, tag="ht", name="ht")
        # per-head Q^T/K^T at base partition 0 (DoubleRow needs row pos 0);
        # full 128 partitions: d 0:64 at slice i=0, zero-pad elsewhere
        qt = persist.tile([P, HPC, 2, N_TOK], FP8, tag="qt", name="qt")
        kt = persist.tile([P, HPC, 2, N_TOK], FP8, tag="kt", name="kt")
        otn = [persist.tile([P, 2, N_TOK], FP8, tag=f"otn{g}", name=f"otn{g}")
               for g in range(2)]
        # cols 0:64 = 1/8, cols 64:128 = V: AV then leaves s/8 broadcast on
        # psum rows 0:64 (base 0), so normalize is approx-recip + mul
        vaug = [persist.tile([P, 2, HPC, P], FP8, tag=f"va{k}", name=f"va{k}")
                for k in range(NKT // 2)]
        # zero-pad / const fills via DMA on the Pool (SWDGE) queue so the
        # SP queue starts x tiles immediately
        nc.gpsimd.dma_start(out=qt[:, :, 1, :], in_=zpad_d[:, :].rearrange(
            "p (h n) -> p h n", h=HPC))
        nc.gpsimd.dma_start(out=kt[:, :, 1, :], in_=zpad_d[:, :].rearrange(
            "p (h n) -> p h n", h=HPC))
        nc.gpsimd.dma_start(out=qt[HD:P, :, 0, :],
                            in_=zpad_d[HD:P, :].rearrange(
                                "p (h n) -> p h n", h=HPC))
        nc.gpsimd.dma_start(out=kt[HD:P, :, 0, :],
                            in_=zpad_d[HD:P, :].rearrange(
                                "p (h n) -> p h n", h=HPC))
        for k in range(NKT // 2):
            nc.gpsimd.dma_start(out=vaug[k][:, :, :, 0:HD],
                                in_=vcst_d[:, :].rearrange(
                                    "p (i h d) -> p i h d", i=2, h=HPC))

        # ---- single shared PSUM pool for transposes/V/QK (via views) + ST --
        with tc.tile_pool(name="ln", bufs=5) as ln_pool, \
             tc.tile_pool(name="lns", bufs=3) as lns, \
             tc.tile_pool(name="expp", bufs=16) as expp, \
             tc.tile_pool(name="att_sm", bufs=4) as att_sm, \
             tc.tile_pool(name="zst", bufs=4) as zst, \
             tc.tile_pool(name="ps_st", bufs=3, space="PSUM") as ps_st, \
             tc.tile_pool(name="ps_ot", bufs=2, space="PSUM") as ps_ot:

            def ps_tile():
                return ps_st.tile([P, 2, 512], F32, tag="st", name="st")

            # ------------- Phase 1: LayerNorm + transpose + V -------------
            for nch in range(4):
                sd4 = lns.tile([P, 4], F32, tag="sd4", name="sd4")
                rstd4 = lns.tile([P, 4], F32, tag="rstd4", name="rstd4")
                nm4 = lns.tile([P, 4], F32, tag="nm4", name="nm4")
                mv4 = lns.tile([P, 4, 2], F32, tag="mv4", name="mv4")
                xts = []
                for t4 in range(4):
                    t = 4 * nch + t4
                    x_t = ln_pool.tile([P, EMB], F32, tag="x", name="x")
                    nc.sync.dma_start(out=x_t, in_=x_d[t * P:(t + 1) * P, :])
                    xts.append(x_t)
                    stats = lns.tile([P, 2, 6], F32, tag="stats", name="stats")
                    nc.vector.bn_stats(out=stats[:, 0, :], in_=x_t[:, 0:512])
                    nc.vector.bn_stats(out=stats[:, 1, :], in_=x_t[:, 512:1024])
                    nc.vector.bn_aggr(out=mv4[:, t4, :], in_=stats)
                    nc.scalar.activation(out=sd4[:, t4:t4 + 1],
                                         in_=mv4[:, t4, 1:2], func=AF.Sqrt,
                                         bias=eps_t, scale=1.0)
                nc.vector.reciprocal(out=rstd4, in_=sd4)
                nc.vector.tensor_scalar_mul(nm4, mv4[:, :, 0], -1.0)
                for t4 in range(4):
                    t = 4 * nch + t4
                    h_t = ln_pool.tile([P, EMB], BF16, tag="h", name="h")
                    nc.gpsimd.tensor_scalar(
                        out=h_t, in0=xts[t4],
                        scalar1=nm4[:, t4:t4 + 1], scalar2=rstd4[:, t4:t4 + 1],
                        op0=mybir.AluOpType.add, op1=mybir.AluOpType.mult)
                    pt = ps_tile().bitcast(BF16)[:, 0, :].rearrange(
                        "p (j m) -> p j m", j=8)
                    for j in range(8):
                        nc.tensor.transpose(pt[:, j, :],
                                            h_t[:, j * P:(j + 1) * P], ident)
                    nc.scalar.copy(out=ht[:, :, t * P:(t + 1) * P], in_=pt)
                    pv = ps_tile()[:, 0, :]
                    for c in range(4):
                        nc.tensor.matmul(
                            pv, lhsT=ht[:, 2 * c:2 * c + 2, t * P:(t + 1) * P],
                            rhs=wv_s[:, :, c, :],
                            start=(c == 0), stop=(c == 3), perf_mode=DR)
                    nc.vector.tensor_scalar_mul(
                        vaug[t // 2][:, t % 2, :, HD:P],
                        pv.rearrange("p (h d) -> p h d", h=HPC), 1.0 / WSCALE)

            # ------------- QK for one head (interleaved below) -------------
            def emit_qk(h):
                for src_w, bias, dst in ((wq_s, bqt, qt), (wk_s, bkt, kt)):
                    for half in range(2):
                        pq = ps_tile()[0:HD, :, :]
                        for n2 in range(2):
                            n = 2 * half + n2
                            for c in range(4):
                                nc.tensor.matmul(
                                    pq[:, n2, :], lhsT=src_w[:, :, h, c, :],
                                    rhs=ht[:, 2 * c:2 * c + 2,
                                           n * 512:(n + 1) * 512],
                                    start=(c == 0), stop=(c == 3),
                                    perf_mode=DR)
                        nc.vector.tensor_scalar(
                            out=dst[0:HD, h, 0, half * 1024:(half + 1) * 1024]
                            .rearrange("p (n m) -> p n m", n=2), in0=pq,
                            scalar1=1.0 / WSCALE, scalar2=bias[:, h:h + 1],
                            op0=mybir.AluOpType.mult, op1=mybir.AluOpType.add)

            def emit_st(h, q):
                """Scores + exp for one (head, q-chunk); returns E tiles."""
                ets = []
                for c in range(NKT // 2):
                    pst = ps_tile()
                    for i in range(2):
                        k = 2 * c + i
                        nc.tensor.matmul(
                            pst[:, i, :],
                            lhsT=kt[:, h, :, k * P:(k + 1) * P],
                            rhs=qt[:, h, :, q * 512:(q + 1) * 512],
                            start=True, stop=True, perf_mode=DR)
                    e_t = expp.tile([P, 2, 512], FP8, tag="e", name="e")
                    # shift by -6 so exp fits fp8e4 range (cancels in softmax)
                    nc.scalar.activation(out=e_t, in_=pst, func=AF.Exp,
                                         bias=nshift)
                    ets.append(e_t)
                return ets

            def emit_av(h, q, ets):
                """att@v + normalize for one (head, q-chunk)."""
                g = h // 4
                pot = ps_ot.tile([P, 512], F32, tag="ot", name="ot")
                for c in range(NKT // 2):
                    nc.tensor.matmul(pot, lhsT=vaug[c][:, :, h, :],
                                     rhs=ets[c],
                                     start=(c == 0), stop=(c == NKT // 2 - 1),
                                     perf_mode=DR)
                # rows 0:64 hold s/8 (const-block of 1/8 in vaug cols 0:64)
                rec_sb = att_sm.tile([HD, 512], F32, tag="rec", name="rec")
                nc.vector.reciprocal_approx_fast(out=rec_sb, in_=pot[0:HD, :])
                nc.vector.tensor_mul(
                    otn[g][64 * (h % 2):64 * (h % 2) + 64, (h % 4) // 2,
                           q * 512:(q + 1) * 512],
                    pot[HD:P, :], rec_sb)

            def emit_proj(t):
                """projection for one token tile (128 tokens x 1024 emb)."""
                for ec in range(2):
                    pz = ps_ot.tile([P, 512], F32, tag="ot", name="z")
                    for g in range(2):
                        nc.tensor.matmul(
                            pz, lhsT=otn[g][:, :, t * P:(t + 1) * P],
                            rhs=wp_s[:, :, g, ec * 512:(ec + 1) * 512],
                            start=(g == 0), stop=(g == 1), perf_mode=DR)
                    z_t = zst.tile([P, 512], BF16, tag="z", name="z")
                    nc.vector.tensor_scalar_mul(z_t, pz, 1.0 / (WSCALE * OSCALE))
                    nc.sync.dma_start(
                        out=z_d[t * P:(t + 1) * P, ec * 512:(ec + 1) * 512],
                        in_=z_t)

            # attention h-outer; QK of head h+1 woven into head h's stream;
            # projection woven into the last head's stream
            emit_qk(0)
            prev = None
            for h in range(HPC):
                for q in range(QCH):
                    ets = emit_st(h, q)
                    if prev is not None:
                        emit_av(*prev)
                        ph, pq_, _ = prev
                        if ph == HPC - 1 and pq_ > 0:
                            for t in range(4 * (pq_ - 1), 4 * pq_):
                                emit_proj(t)
                    prev = (h, q, ets)
                    if q == 0 and h + 1 < HPC:
                        emit_qk(h + 1)
            emit_av(*prev)
            for t in range(4 * (QCH - 2), NT):
                emit_proj(t)


_CACHE = {}


def _get_nc():
    if "nc" not in _CACHE:
        _CACHE["nc"] = build_nc()
    return _CACHE["nc"]


def _prep_in_maps(x, ln_w, ln_b, w_qkv, b_qkv, w_proj, b_proj):
    bf = ml_dtypes.bfloat16
    f8 = ml_dtypes.float8_e4m3fn
    x = np.asarray(x, np.float32)
    ln_w = np.asarray(ln_w, np.float32)
    ln_b = np.asarray(ln_b, np.float32)
    w_qkv = np.asarray(w_qkv, np.float32)
    b_qkv = np.asarray(b_qkv, np.float32)
    w_proj = np.asarray(w_proj, np.float32)
    b_proj = np.asarray(b_proj, np.float32)

    b_eff = b_qkv + ln_b @ w_qkv
    w_eff = ln_w[:, None] * w_qkv
    w4 = w_eff.reshape(EMB, HEADS, HD, 3)
    b4 = b_eff.reshape(HEADS, HD, 3)
    wq = w4[..., 0] * SCALE
    wk = w4[..., 1]
    wv = w4[..., 2]
    bq = b4[..., 0] * SCALE
    bk = b4[..., 1]
    bv = b4[..., 2]

    def pack_qk(w, hsl):
        # w [EMB, 8 heads, 64] -> [128p, 2i, 8h, 4c, 64d] fp8 (x WSCALE)
        # emb = 256c + 128i + p
        wh = w[:, hsl, :]                                    # [1024, 8, 64]
        wh = wh.reshape(4, 2, P, HPC, HD)                    # c i p h d
        wh = wh.transpose(2, 1, 3, 0, 4)                     # p i h c d
        return np.ascontiguousarray((wh * WSCALE).reshape(P, -1)).astype(f8)

    def pack_qk_bias(b, hsl):
        # b [8 heads, 64] -> [64d, 8h] f32
        return np.ascontiguousarray(b[hsl].T.astype(np.float32))

    def pack_v(w, hsl):
        # w [EMB, 8, 64] -> [128p, 2i, 4c, 512hd] fp8 (x WSCALE)
        wh = w[:, hsl, :].reshape(4, 2, P, 512)              # c i p hd
        wh = wh.transpose(2, 1, 0, 3)                        # p i c hd
        return np.ascontiguousarray((wh * WSCALE).reshape(P, -1)).astype(f8)

    def pack_wp(w, hg):
        # w_proj rows for this head group [512, 1024] -> [128p, 2i, 2g, 1024]
        wh = w[hg * 512:(hg + 1) * 512, :]                   # hd=256g+128i+p
        wh = wh.reshape(2, 2, P, EMB).transpose(2, 1, 0, 3)  # p i g n
        return np.ascontiguousarray((wh * WSCALE).reshape(P, -1)).astype(f8)

    f8z = np.zeros((P, 8 * N_TOK), f8)
    vc = np.full((P, 2 * 8 * HD), 1.0 / OSCALE, f8)
    in_maps = []
    for cid in range(N_CORES):
        bi, hg = divmod(cid, 2)
        hsl = slice(hg * HPC, (hg + 1) * HPC)
        in_maps.append({
            "x": np.ascontiguousarray(x[bi]),
            "wq": pack_qk(wq, hsl),
            "wk": pack_qk(wk, hsl),
            "wv": pack_v(wv, hsl),
            "wp": pack_wp(w_proj, hg),
            "bqt": pack_qk_bias(bq, hsl),
            "bkt": pack_qk_bias(bk, hsl),
            "zpad": f8z,
            "vcst": vc,
        })
    return in_maps


def _gather(results, x, b_proj_eff):
    x = np.asarray(x, np.float32)
    out = np.empty((x.shape[0], N_TOK, EMB), np.float32)
    for bi in range(x.shape[0]):
        out[bi] = (results[2 * bi]["z"].astype(np.float32)
                   + results[2 * bi + 1]["z"].astype(np.float32)
                   + b_proj_eff[None, :] + x[bi])
    return out


def _run(inputs, **kw):
    in_maps = _prep_in_maps(**inputs)
    # exact fold of V bias into projection bias
    b_eff = np.asarray(inputs["b_qkv"], np.float32) + \
        np.asarray(inputs["ln_b"], np.float32) @ np.asarray(
            inputs["w_qkv"], np.float32)
    bv = b_eff.reshape(HEADS, HD, 3)[..., 2].reshape(HEADS * HD)
    b_proj_eff = np.asarray(inputs["b_proj"], np.float32) + \
        bv @ np.asarray(inputs["w_proj"], np.float32)
    res = run_bass_kernel_spmd(_get_nc(), in_maps,
                               core_ids=list(range(N_CORES)), **kw)
    out = _gather(res.results, inputs["x"], b_proj_eff)
    return out, res


def kernel(**inputs):
    out, _ = _run(inputs)
    return out


# revision 61
# speedup vs baseline: 1.7189x; 1.0010x over previous
"""Trainium2 Bass kernel for a pre-LN multi-head attention block.

Full-input contract: kernel(**inputs) takes the unsharded tensors from
setup_inputs() and returns the full [4, 2048, 1024] output.

Sharding: 8 cores = 4 batches x 2 head-groups (8 heads each).
Each core computes LayerNorm(x[b]), its 8 heads of QKV + attention, and a
partial projection.  Host sums the two partials per batch and adds
b_proj_eff + residual.

Numerics strategy: all matmuls run in fp8e4 (e4m3) DoubleRow perf mode
(2 contraction sub-rows per instruction at 0.5 cycles/row).  Weights are
pre-scaled x32 into fp8's representable range host-side; the 1/32 is
folded into the PSUM->SBUF cast passes.  The attention path is only ~8%
of the output norm (residual dominates), so fp8's ~6% element error
lands at ~2e-3 total relative error, well under tolerance.

Host-side algebraic folds (exact):
  - ln_w folded into w_qkv columns, ln_b folded into b_qkv
  - softmax scale folded into W_q / b_q
  - V bias folded into b_proj (attention rows sum to 1)

Layouts (DoubleRow pairs contraction rows along a middle dim of 2):
  ht    [128, 8, 2048] fp8: ht[p, 2c+i, t] = h[t, emb=256c+128i+p]
  qt/kt [g][128, 2, 2048] fp8: [32s+j, r, t] = Q[t, head 4g+s, d=32r+j]
  vaug  [kk][128, 2, 8, 65] fp8: [p, i, h, d] = V[tok=256kk+128i+p, h, d],
        d=64 column = 1.0 (softmax-sum row trick)
  E     [128, 2, 512] fp8 per (k-pair, q-chunk): [p, i, q]=exp(S[k,q])
  otn   [g][128, 2, 2048] fp8: [p, i, t] = 8*OT_norm[hd=256g+128i+p, t]
"""

import sys

sys.path.insert(0, "/opt/trn_rl_repo")

import numpy as np
import ml_dtypes

import concourse.bass as bass
from concourse import bacc
import concourse.tile as tile
from concourse import mybir
from concourse.bass_utils import run_bass_kernel_spmd
from concourse.masks import make_identity

EMB = 1024
HEADS = 16
HD = 64
SCALE = HD ** -0.5
N_TOK = 2048
N_CORES = 8
HPC = 8                 # heads per core
QK_COLS = HPC * HD      # 512
P = 128
NT = N_TOK // P         # 16 token tiles
QCH = 4                 # q chunks of 512
NKT = 16                # k tiles of 128
WSCALE = 32.0           # host pre-scale on all weights (fp8 range)
OSCALE = 8.0            # extra scale on normalized attn output

BF16 = mybir.dt.bfloat16
F32 = mybir.dt.float32
FP8 = mybir.dt.float8e4
AF = mybir.ActivationFunctionType
DR = mybir.MatmulPerfMode.DoubleRow


def build_nc():
    nc = bacc.Bacc(trn_type="TRN2", target_bir_lowering=False)

    x_d = nc.dram_tensor("x", [N_TOK, EMB], F32, kind="ExternalInput")
    wq_d = nc.dram_tensor("wq", [P, 2 * 8 * 4 * HD], FP8, kind="ExternalInput")
    wk_d = nc.dram_tensor("wk", [P, 2 * 8 * 4 * HD], FP8, kind="ExternalInput")
    wv_d = nc.dram_tensor("wv", [P, 2 * 4 * 512], FP8, kind="ExternalInput")
    wp_d = nc.dram_tensor("wp", [P, 2 * 2 * EMB], FP8, kind="ExternalInput")
    bqt_d = nc.dram_tensor("bqt", [HD, HPC], F32, kind="ExternalInput")
    bkt_d = nc.dram_tensor("bkt", [HD, HPC], F32, kind="ExternalInput")
    zpad_d = nc.dram_tensor("zpad", [P, 8 * N_TOK], FP8, kind="ExternalInput")
    vcst_d = nc.dram_tensor("vcst", [P, 2 * 8 * HD], FP8, kind="ExternalInput")
    z_d = nc.dram_tensor("z", [N_TOK, EMB], BF16, kind="ExternalOutput")

    with tile.TileContext(nc) as tc:
        _emit(nc, tc, x_d, wq_d, wk_d, wv_d, wp_d, bqt_d, bkt_d,
              zpad_d, vcst_d, z_d)
    nc.finalize()
    return nc


def _emit(nc, tc, x_d, wq_d, wk_d, wv_d, wp_d, bqt_d, bkt_d,
          zpad_d, vcst_d, z_d):
    from contextlib import ExitStack

    ctx = ExitStack()
    with ctx:
        consts = ctx.enter_context(tc.tile_pool(name="consts", bufs=1))
        persist = ctx.enter_context(tc.tile_pool(name="persist", bufs=1))

        eps_t = consts.tile([P, 1], F32, tag="eps", name="eps")
        nc.vector.memset(eps_t, 1e-5)
        nshift = consts.tile([P, 1], F32, tag="nshift", name="nshift")
        nc.vector.memset(nshift, -6.0)

        # weights: wq/wk as [128, 2(i), 8(h), 4(c), 64(m=d)]
        wq_s = persist.tile([P, 2, 8, 4, HD], FP8, tag="wq", name="wq")
        nc.sync.dma_start(out=wq_s, in_=wq_d[:, :].rearrange(
            "p (i h c m) -> p i h c m", i=2, h=8, c=4))
        wk_s = persist.tile([P, 2, 8, 4, HD], FP8, tag="wk", name="wk")
        nc.sync.dma_start(out=wk_s, in_=wk_d[:, :].rearrange(
            "p (i h c m) -> p i h c m", i=2, h=8, c=4))
        # wv as [128, 2(i), 4(c), 512(hd)]
        wv_s = persist.tile([P, 2, 4, 512], FP8, tag="wv", name="wv")
        nc.sync.dma_start(out=wv_s, in_=wv_d[:, :].rearrange(
            "p (i c m) -> p i c m", i=2, c=4))
        # wp as [128, 2(i), 2(g), 1024(n)]
        wp_s = persist.tile([P, 2, 2, EMB], FP8, tag="wp", name="wp")
        nc.sync.dma_start(out=wp_s, in_=wp_d[:, :].rearrange(
            "p (i g n) -> p i g n", i=2, g=2))
        bqt = consts.tile([HD, HPC], F32, tag="bqt", name="bqt")
        nc.sync.dma_start(out=bqt, in_=bqt_d[:, :])
        bkt = consts.tile([HD, HPC], F32, tag="bkt", name="bkt")
        nc.sync.dma_start(out=bkt, in_=bkt_d[:, :])

        ident = consts.tile([P, P], BF16, tag="ident", name="ident")
        make_identity(nc, ident)

        ht = persist.tile([P, 8, N_TOK], FP8, tag="ht", name="ht")
        # per-head Q^T/K^T at base partition 0 (DoubleRow needs row pos 0);
        # 64 partitions, DR slice i=1 zero-padded (halves ldweights rows)
        qt = persist.tile([HD, HPC, 2, N_TOK], FP8, tag="qt", name="qt")
        kt = persist.tile([HD, HPC, 2, N_TOK], FP8, tag="kt", name="kt")
        otn = [persist.tile([P, 2, N_TOK], FP8, tag=f"otn{g}", name=f"otn{g}")
               for g in range(2)]
        # cols 0:64 = 1/8, cols 64:128 = V: AV then leaves s/8 broadcast on
        # psum rows 0:64 (base 0), so normalize is approx-recip + mul
        vaug = [persist.tile([P, 2, HPC, P], FP8, tag=f"va{k}", name=f"va{k}")
                for k in range(NKT // 2)]
        # zero-pad / const fills via DMA on the Pool (SWDGE) queue so the
        # SP queue starts x tiles immediately
        nc.gpsimd.dma_start(out=qt[:, :, 1, :],
                            in_=zpad_d[0:HD, :].rearrange(
                                "p (h n) -> p h n", h=HPC))
        nc.gpsimd.dma_start(out=kt[:, :, 1, :],
                            in_=zpad_d[0:HD, :].rearrange(
                                "p (h n) -> p h n", h=HPC))
        for k in range(NKT // 2):
            nc.gpsimd.dma_start(out=vaug[k][:, :, :, 0:HD],
                                in_=vcst_d[:, :].rearrange(
                                    "p (i h d) -> p i h d", i=2, h=HPC))

        # ---- single shared PSUM pool for transposes/V/QK (via views) + ST --
        with tc.tile_pool(name="ln", bufs=5) as ln_pool, \
             tc.tile_pool(name="lns", bufs=3) as lns, \
             tc.tile_pool(name="expp", bufs=24) as expp, \
             tc.tile_pool(name="att_sm", bufs=6) as att_sm, \
             tc.tile_pool(name="zst", bufs=4) as zst, \
             tc.tile_pool(name="ps_st", bufs=3, space="PSUM") as ps_st, \
             tc.tile_pool(name="ps_ot", bufs=2, space="PSUM") as ps_ot:

            def ps_tile():
                return ps_st.tile([P, 2, 512], F32, tag="st", name="st")

            # ------------- Phase 1: LayerNorm + transpose + V -------------
            for nch in range(4):
                sd4 = lns.tile([P, 4], F32, tag="sd4", name="sd4")
                rstd4 = lns.tile([P, 4], F32, tag="rstd4", name="rstd4")
                nm4 = lns.tile([P, 4], F32, tag="nm4", name="nm4")
                mv4 = lns.tile([P, 4, 2], F32, tag="mv4", name="mv4")
                xts = []
                for t4 in range(4):
                    t = 4 * nch + t4
                    x_t = ln_pool.tile([P, EMB], F32, tag="x", name="x")
                    nc.sync.dma_start(out=x_t, in_=x_d[t * P:(t + 1) * P, :])
                    xts.append(x_t)
                    stats = lns.tile([P, 2, 6], F32, tag="stats", name="stats")
                    nc.vector.bn_stats(out=stats[:, 0, :], in_=x_t[:, 0:512])
                    nc.vector.bn_stats(out=stats[:, 1, :], in_=x_t[:, 512:1024])
                    nc.vector.bn_aggr(out=mv4[:, t4, :], in_=stats)
                    nc.scalar.activation(out=sd4[:, t4:t4 + 1],
                                         in_=mv4[:, t4, 1:2], func=AF.Sqrt,
                                         bias=eps_t, scale=1.0)
                nc.vector.reciprocal(out=rstd4, in_=sd4)
                nc.vector.tensor_scalar_mul(nm4, mv4[:, :, 0], -1.0)
                for t4 in range(4):
                    t = 4 * nch + t4
                    h_t = ln_pool.tile([P, EMB], BF16, tag="h", name="h")
                    nc.gpsimd.tensor_scalar(
                        out=h_t, in0=xts[t4],
                        scalar1=nm4[:, t4:t4 + 1], scalar2=rstd4[:, t4:t4 + 1],
                        op0=mybir.AluOpType.add, op1=mybir.AluOpType.mult)
                    pt = ps_tile().bitcast(BF16)[:, 0, :].rearrange(
                        "p (j m) -> p j m", j=8)
                    for j in range(8):
                        nc.tensor.transpose(pt[:, j, :],
                                            h_t[:, j * P:(j + 1) * P], ident)
                    nc.scalar.copy(out=ht[:, :, t * P:(t + 1) * P], in_=pt)
                    pv = ps_tile()[:, 0, :]
                    for c in range(4):
                        nc.tensor.matmul(
                            pv, lhsT=ht[:, 2 * c:2 * c + 2, t * P:(t + 1) * P],
                            rhs=wv_s[:, :, c, :],
                            start=(c == 0), stop=(c == 3), perf_mode=DR)
                    nc.vector.tensor_scalar_mul(
                        vaug[t // 2][:, t % 2, :, HD:P],
                        pv.rearrange("p (h d) -> p h d", h=HPC), 1.0 / WSCALE)

            # ------------- QK for one head (interleaved below) -------------
            def emit_qk(h):
                for src_w, bias, dst in ((wq_s, bqt, qt), (wk_s, bkt, kt)):
                    for half in range(2):
                        pq = ps_tile()[0:HD, :, :]
                        for n2 in range(2):
                            n = 2 * half + n2
                            for c in range(4):
                                nc.tensor.matmul(
                                    pq[:, n2, :], lhsT=src_w[:, :, h, c, :],
                                    rhs=ht[:, 2 * c:2 * c + 2,
                                           n * 512:(n + 1) * 512],
                                    start=(c == 0), stop=(c == 3),
                                    perf_mode=DR)
                        nc.vector.tensor_scalar(
                            out=dst[:, h, 0, half * 1024:(half + 1) * 1024]
                            .rearrange("p (n m) -> p n m", n=2), in0=pq,
                            scalar1=1.0 / WSCALE, scalar2=bias[:, h:h + 1],
                            op0=mybir.AluOpType.mult, op1=mybir.AluOpType.add)

            def emit_st(h, q):
                """Scores + exp for one (head, q-chunk); returns E tiles."""
                ets = []
                for c in range(NKT // 2):
                    pst = ps_tile()
                    for i in range(2):
                        k = 2 * c + i
                        nc.tensor.matmul(
                            pst[:, i, :],
                            lhsT=kt[:, h, :, k * P:(k + 1) * P],
                            rhs=qt[:, h, :, q * 512:(q + 1) * 512],
                            start=True, stop=True, perf_mode=DR)
                    e_t = expp.tile([P, 2, 512], FP8, tag="e", name="e")
                    # shift by -6 so exp fits fp8e4 range (cancels in softmax)
                    nc.scalar.activation(out=e_t, in_=pst, func=AF.Exp,
                                         bias=nshift)
                    ets.append(e_t)
                return ets

            def emit_av(h, q, ets):
                """att@v + normalize for one (head, q-chunk)."""
                g = h // 4
                pot = ps_ot.tile([P, 512], F32, tag="ot", name="ot")
                for c in range(NKT // 2):
                    nc.tensor.matmul(pot, lhsT=vaug[c][:, :, h, :],
                                     rhs=ets[c],
                                     start=(c == 0), stop=(c == NKT // 2 - 1),
                                     perf_mode=DR)
                # rows 0:64 hold s/8 (const-block of 1/8 in vaug cols 0:64)
                rec_sb = att_sm.tile([HD, 512], F32, tag="rec", name="rec")
                nc.vector.reciprocal_approx_fast(out=rec_sb, in_=pot[0:HD, :])
                nc.vector.tensor_mul(
                    otn[g][64 * (h % 2):64 * (h % 2) + 64, (h % 4) // 2,
                           q * 512:(q + 1) * 512],
                    pot[HD:P, :], rec_sb)

            def emit_proj(t):
                """projection for one token tile (128 tokens x 1024 emb)."""
                for ec in range(2):
                    pz = ps_ot.tile([P, 512], F32, tag="ot", name="z")
                    for g in range(2):
                        nc.tensor.matmul(
                            pz, lhsT=otn[g][:, :, t * P:(t + 1) * P],
                            rhs=wp_s[:, :, g, ec * 512:(ec + 1) * 512],
                            start=(g == 0), stop=(g == 1), perf_mode=DR)
                    z_t = zst.tile([P, 512], BF16, tag="z", name="z")
                    nc.vector.tensor_scalar_mul(z_t, pz, 1.0 / (WSCALE * OSCALE))
                    nc.sync.dma_start(
                        out=z_d[t * P:(t + 1) * P, ec * 512:(ec + 1) * 512],
                        in_=z_t)

            # attention h-outer; QK of head h+1 woven into head h's stream;
            # projection woven into the last head's stream
            emit_qk(0)
            prev = None
            for h in range(HPC):
                for q in range(QCH):
                    ets = emit_st(h, q)
                    if prev is not None:
                        emit_av(*prev)
                        ph, pq_, _ = prev
                        if ph == HPC - 1 and pq_ > 0:
                            for t in range(4 * (pq_ - 1), 4 * pq_):
                                emit_proj(t)
                    prev = (h, q, ets)
                    if q == 0 and h + 1 < HPC:
                        emit_qk(h + 1)
            emit_av(*prev)
            for t in range(4 * (QCH - 2), NT):
                emit_proj(t)


_CACHE = {}


def _get_nc():
    if "nc" not in _CACHE:
        _CACHE["nc"] = build_nc()
    return _CACHE["nc"]


def _prep_in_maps(x, ln_w, ln_b, w_qkv, b_qkv, w_proj, b_proj):
    bf = ml_dtypes.bfloat16
    f8 = ml_dtypes.float8_e4m3fn
    x = np.asarray(x, np.float32)
    ln_w = np.asarray(ln_w, np.float32)
    ln_b = np.asarray(ln_b, np.float32)
    w_qkv = np.asarray(w_qkv, np.float32)
    b_qkv = np.asarray(b_qkv, np.float32)
    w_proj = np.asarray(w_proj, np.float32)
    b_proj = np.asarray(b_proj, np.float32)

    b_eff = b_qkv + ln_b @ w_qkv
    w_eff = ln_w[:, None] * w_qkv
    w4 = w_eff.reshape(EMB, HEADS, HD, 3)
    b4 = b_eff.reshape(HEADS, HD, 3)
    wq = w4[..., 0] * SCALE
    wk = w4[..., 1]
    wv = w4[..., 2]
    bq = b4[..., 0] * SCALE
    bk = b4[..., 1]
    bv = b4[..., 2]

    def pack_qk(w, hsl):
        # w [EMB, 8 heads, 64] -> [128p, 2i, 8h, 4c, 64d] fp8 (x WSCALE)
        # emb = 256c + 128i + p
        wh = w[:, hsl, :]                                    # [1024, 8, 64]
        wh = wh.reshape(4, 2, P, HPC, HD)                    # c i p h d
        wh = wh.transpose(2, 1, 3, 0, 4)                     # p i h c d
        return np.ascontiguousarray((wh * WSCALE).reshape(P, -1)).astype(f8)

    def pack_qk_bias(b, hsl):
        # b [8 heads, 64] -> [64d, 8h] f32
        return np.ascontiguousarray(b[hsl].T.astype(np.float32))

    def pack_v(w, hsl):
        # w [EMB, 8, 64] -> [128p, 2i, 4c, 512hd] fp8 (x WSCALE)
        wh = w[:, hsl, :].reshape(4, 2, P, 512)              # c i p hd
        wh = wh.transpose(2, 1, 0, 3)                        # p i c hd
        return np.ascontiguousarray((wh * WSCALE).reshape(P, -1)).astype(f8)

    def pack_wp(w, hg):
        # w_proj rows for this head group [512, 1024] -> [128p, 2i, 2g, 1024]
        wh = w[hg * 512:(hg + 1) * 512, :]                   # hd=256g+128i+p
        wh = wh.reshape(2, 2, P, EMB).transpose(2, 1, 0, 3)  # p i g n
        return np.ascontiguousarray((wh * WSCALE).reshape(P, -1)).astype(f8)

    f8z = np.zeros((P, 8 * N_TOK), f8)
    vc = np.full((P, 2 * 8 * HD), 1.0 / OSCALE, f8)
    in_maps = []
    for cid in range(N_CORES):
        bi, hg = divmod(cid, 2)
        hsl = slice(hg * HPC, (hg + 1) * HPC)
        in_maps.append({
            "x": np.ascontiguousarray(x[bi]),
            "wq": pack_qk(wq, hsl),
            "wk": pack_qk(wk, hsl),
            "wv": pack_v(wv, hsl),
            "wp": pack_wp(w_proj, hg),
            "bqt": pack_qk_bias(bq, hsl),
            "bkt": pack_qk_bias(bk, hsl),
            "zpad": f8z,
            "vcst": vc,
        })
    return in_maps


def _gather(results, x, b_proj_eff):
    x = np.asarray(x, np.float32)
    out = np.empty((x.shape[0], N_TOK, EMB), np.float32)
    for bi in range(x.shape[0]):
        out[bi] = (results[2 * bi]["z"].astype(np.float32)
                   + results[2 * bi + 1]["z"].astype(np.float32)
                   + b_proj_eff[None, :] + x[bi])
    return out


def _run(inputs, **kw):
    in_maps = _prep_in_maps(**inputs)
    # exact fold of V bias into projection bias
    b_eff = np.asarray(inputs["b_qkv"], np.float32) + \
        np.asarray(inputs["ln_b"], np.float32) @ np.asarray(
            inputs["w_qkv"], np.float32)
    bv = b_eff.reshape(HEADS, HD, 3)[..., 2].reshape(HEADS * HD)
    b_proj_eff = np.asarray(inputs["b_proj"], np.float32) + \
        bv @ np.asarray(inputs["w_proj"], np.float32)
    res = run_bass_kernel_spmd(_get_nc(), in_maps,
                               core_ids=list(range(N_CORES)), **kw)
    out = _gather(res.results, inputs["x"], b_proj_eff)
    return out, res


def kernel(**inputs):
    out, _ = _run(inputs)
    return out


# revision 64
# speedup vs baseline: 1.7342x; 1.0089x over previous
"""Trainium2 Bass kernel for a pre-LN multi-head attention block.

Full-input contract: kernel(**inputs) takes the unsharded tensors from
setup_inputs() and returns the full [4, 2048, 1024] output.

Sharding: 8 cores = 4 batches x 2 head-groups (8 heads each).
Each core computes LayerNorm(x[b]), its 8 heads of QKV + attention, and a
partial projection.  Host sums the two partials per batch and adds
b_proj_eff + residual.

Numerics strategy: all matmuls run in fp8e4 (e4m3) DoubleRow perf mode
(2 contraction sub-rows per instruction at 0.5 cycles/row).  Weights are
pre-scaled x32 into fp8's representable range host-side; the 1/32 is
folded into the PSUM->SBUF cast passes.  The attention path is only ~8%
of the output norm (residual dominates), so fp8's ~6% element error
lands at ~2e-3 total relative error, well under tolerance.

Host-side algebraic folds (exact):
  - ln_w folded into w_qkv columns, ln_b folded into b_qkv
  - softmax scale folded into W_q / b_q
  - V bias folded into b_proj (attention rows sum to 1)

Layouts (DoubleRow pairs contraction rows along a middle dim of 2):
  ht    [128, 8, 2048] fp8: ht[p, 2c+i, t] = h[t, emb=256c+128i+p]
  qt/kt [g][128, 2, 2048] fp8: [32s+j, r, t] = Q[t, head 4g+s, d=32r+j]
  vaug  [kk][128, 2, 8, 65] fp8: [p, i, h, d] = V[tok=256kk+128i+p, h, d],
        d=64 column = 1.0 (softmax-sum row trick)
  E     [128, 2, 512] fp8 per (k-pair, q-chunk): [p, i, q]=exp(S[k,q])
  otn   [g][128, 2, 2048] fp8: [p, i, t] = 8*OT_norm[hd=256g+128i+p, t]
"""

import sys

sys.path.insert(0, "/opt/trn_rl_repo")

import numpy as np
import ml_dtypes

import concourse.bass as bass
from concourse import bacc
import concourse.tile as tile
from concourse import mybir
from concourse.bass_utils import run_bass_kernel_spmd
from concourse.masks import make_identity

EMB = 1024
HEADS = 16
HD = 64
SCALE = HD ** -0.5
N_TOK = 2048
N_CORES = 8
HPC = 8                 # heads per core
QK_COLS = HPC * HD      # 512
P = 128
NT = N_TOK // P         # 16 token tiles
QCH = 4                 # q chunks of 512
NKT = 16                # k tiles of 128
WSCALE = 32.0           # host pre-scale on all weights (fp8 range)
OSCALE = 8.0            # extra scale on normalized attn output

BF16 = mybir.dt.bfloat16
F32 = mybir.dt.float32
FP8 = mybir.dt.float8e4
AF = mybir.ActivationFunctionType
DR = mybir.MatmulPerfMode.DoubleRow


def build_nc():
    nc = bacc.Bacc(trn_type="TRN2", target_bir_lowering=False)

    x_d = nc.dram_tensor("x", [N_TOK, EMB], F32, kind="ExternalInput")
    wq_d = nc.dram_tensor("wq", [P, 2 * 8 * 4 * HD], FP8, kind="ExternalInput")
    wk_d = nc.dram_tensor("wk", [P, 2 * 8 * 4 * HD], FP8, kind="ExternalInput")
    wv_d = nc.dram_tensor("wv", [P, 2 * 4 * 512], FP8, kind="ExternalInput")
    wp_d = nc.dram_tensor("wp", [P, 2 * 2 * EMB], FP8, kind="ExternalInput")
    bqt_d = nc.dram_tensor("bqt", [HD, HPC], F32, kind="ExternalInput")
    bkt_d = nc.dram_tensor("bkt", [HD, HPC], F32, kind="ExternalInput")
    zpad_d = nc.dram_tensor("zpad", [P, 8 * N_TOK], FP8, kind="ExternalInput")
    vcst_d = nc.dram_tensor("vcst", [P, 2 * 8 * HD], FP8, kind="ExternalInput")
    z_d = nc.dram_tensor("z", [N_TOK, EMB], BF16, kind="ExternalOutput")

    with tile.TileContext(nc) as tc:
        _emit(nc, tc, x_d, wq_d, wk_d, wv_d, wp_d, bqt_d, bkt_d,
              zpad_d, vcst_d, z_d)
    nc.finalize()
    return nc


def _emit(nc, tc, x_d, wq_d, wk_d, wv_d, wp_d, bqt_d, bkt_d,
          zpad_d, vcst_d, z_d):
    from contextlib import ExitStack

    ctx = ExitStack()
    with ctx:
        consts = ctx.enter_context(tc.tile_pool(name="consts", bufs=1))
        persist = ctx.enter_context(tc.tile_pool(name="persist", bufs=1))

        eps_t = consts.tile([P, 1], F32, tag="eps", name="eps")
        nc.vector.memset(eps_t, 1e-5)
        nshift = consts.tile([P, 1], F32, tag="nshift", name="nshift")
        nc.vector.memset(nshift, -6.0)

        # weights: wq/wk as [128, 2(i), 8(h), 4(c), 64(m=d)]
        wq_s = persist.tile([P, 2, 8, 4, HD], FP8, tag="wq", name="wq")
        nc.sync.dma_start(out=wq_s, in_=wq_d[:, :].rearrange(
            "p (i h c m) -> p i h c m", i=2, h=8, c=4))
        wk_s = persist.tile([P, 2, 8, 4, HD], FP8, tag="wk", name="wk")
        nc.sync.dma_start(out=wk_s, in_=wk_d[:, :].rearrange(
            "p (i h c m) -> p i h c m", i=2, h=8, c=4))
        # wv as [128, 2(i), 4(c), 512(hd)]
        wv_s = persist.tile([P, 2, 4, 512], FP8, tag="wv", name="wv")
        nc.sync.dma_start(out=wv_s, in_=wv_d[:, :].rearrange(
            "p (i c m) -> p i c m", i=2, c=4))
        # wp as [128, 2(i), 2(g), 1024(n)]
        wp_s = persist.tile([P, 2, 2, EMB], FP8, tag="wp", name="wp")
        nc.sync.dma_start(out=wp_s, in_=wp_d[:, :].rearrange(
            "p (i g n) -> p i g n", i=2, g=2))
        bqt = consts.tile([HD, HPC], F32, tag="bqt", name="bqt")
        nc.sync.dma_start(out=bqt, in_=bqt_d[:, :])
        bkt = consts.tile([HD, HPC], F32, tag="bkt", name="bkt")
        nc.sync.dma_start(out=bkt, in_=bkt_d[:, :])

        ident = consts.tile([P, P], BF16, tag="ident", name="ident")
        make_identity(nc, ident)

        ht = persist.tile([P, 8, N_TOK], FP8, tag="ht", name="ht")
        # per-head Q^T/K^T at base partition 0 (DoubleRow needs row pos 0);
        # 64 partitions, DR slice i=1 zero-padded (halves ldweights rows)
        qt = persist.tile([HD, HPC, 2, N_TOK], FP8, tag="qt", name="qt")
        kt = persist.tile([HD, HPC, 2, N_TOK], FP8, tag="kt", name="kt")
        otn = [persist.tile([P, 2, N_TOK], FP8, tag=f"otn{g}", name=f"otn{g}")
               for g in range(2)]
        # cols 0:64 = 1/8, cols 64:128 = V: AV then leaves s/8 broadcast on
        # psum rows 0:64 (base 0), so normalize is approx-recip + mul
        vaug = [persist.tile([P, 2, HPC, P], FP8, tag=f"va{k}", name=f"va{k}")
                for k in range(NKT // 2)]
        # zero-pad / const fills via DMA on the Pool (SWDGE) queue so the
        # SP queue starts x tiles immediately
        nc.gpsimd.dma_start(out=qt[:, :, 1, :],
                            in_=zpad_d[0:HD, :].rearrange(
                                "p (h n) -> p h n", h=HPC))
        nc.gpsimd.dma_start(out=kt[:, :, 1, :],
                            in_=zpad_d[0:HD, :].rearrange(
                                "p (h n) -> p h n", h=HPC))
        for k in range(NKT // 2):
            nc.gpsimd.dma_start(out=vaug[k][:, :, :, 0:HD],
                                in_=vcst_d[:, :].rearrange(
                                    "p (i h d) -> p i h d", i=2, h=HPC))

        # ---- single shared PSUM pool for transposes/V/QK (via views) + ST --
        with tc.tile_pool(name="ln", bufs=5) as ln_pool, \
             tc.tile_pool(name="lns", bufs=3) as lns, \
             tc.tile_pool(name="expp", bufs=24) as expp, \
             tc.tile_pool(name="att_sm", bufs=6) as att_sm, \
             tc.tile_pool(name="zst", bufs=4) as zst, \
             tc.tile_pool(name="ps_st", bufs=3, space="PSUM") as ps_st, \
             tc.tile_pool(name="ps_ot", bufs=2, space="PSUM") as ps_ot:

            def ps_tile():
                return ps_st.tile([P, 2, 512], F32, tag="st", name="st")

            # ------------- Phase 1: LayerNorm + transpose + V -------------
            for nch in range(4):
                sd4 = lns.tile([P, 4], F32, tag="sd4", name="sd4")
                rstd4 = lns.tile([P, 4], F32, tag="rstd4", name="rstd4")
                nm4 = lns.tile([P, 4], F32, tag="nm4", name="nm4")
                mv4 = lns.tile([P, 4, 2], F32, tag="mv4", name="mv4")
                xts = []
                for t4 in range(4):
                    t = 4 * nch + t4
                    x_t = ln_pool.tile([P, EMB], F32, tag="x", name="x")
                    nc.sync.dma_start(out=x_t, in_=x_d[t * P:(t + 1) * P, :])
                    xts.append(x_t)
                    stats = lns.tile([P, 2, 6], F32, tag="stats", name="stats")
                    nc.vector.bn_stats(out=stats[:, 0, :], in_=x_t[:, 0:512])
                    nc.vector.bn_stats(out=stats[:, 1, :], in_=x_t[:, 512:1024])
                    nc.vector.bn_aggr(out=mv4[:, t4, :], in_=stats)
                    nc.scalar.activation(out=sd4[:, t4:t4 + 1],
                                         in_=mv4[:, t4, 1:2], func=AF.Sqrt,
                                         bias=eps_t, scale=1.0)
                nc.vector.reciprocal(out=rstd4, in_=sd4)
                nc.vector.tensor_scalar_mul(nm4, mv4[:, :, 0], -1.0)
                for t4 in range(4):
                    t = 4 * nch + t4
                    h_t = ln_pool.tile([P, EMB], BF16, tag="h", name="h")
                    nc.gpsimd.tensor_scalar(
                        out=h_t, in0=xts[t4],
                        scalar1=nm4[:, t4:t4 + 1], scalar2=rstd4[:, t4:t4 + 1],
                        op0=mybir.AluOpType.add, op1=mybir.AluOpType.mult)
                    pt = ps_tile().bitcast(BF16)[:, 0, :].rearrange(
                        "p (j m) -> p j m", j=8)
                    for j in range(8):
                        nc.tensor.transpose(pt[:, j, :],
                                            h_t[:, j * P:(j + 1) * P], ident)
                    nc.vector.tensor_copy(out=ht[:, :, t * P:(t + 1) * P],
                                          in_=pt)
                    pv = ps_tile()[:, 0, :]
                    for c in range(4):
                        nc.tensor.matmul(
                            pv, lhsT=ht[:, 2 * c:2 * c + 2, t * P:(t + 1) * P],
                            rhs=wv_s[:, :, c, :],
                            start=(c == 0), stop=(c == 3), perf_mode=DR)
                    nc.vector.tensor_scalar_mul(
                        vaug[t // 2][:, t % 2, :, HD:P],
                        pv.rearrange("p (h d) -> p h d", h=HPC), 1.0 / WSCALE)

            # ---- QK for one head, one quarter (unit) at a time: woven into
            # the attention stream so ST production never pauses long ----
            def emit_qk_unit(h, u):
                src_w, bias, dst = ((wq_s, bqt, qt), (wk_s, bkt, kt))[u // 2]
                half = u % 2
                pq = ps_tile()[0:HD, :, :]
                for n2 in range(2):
                    n = 2 * half + n2
                    for c in range(4):
                        nc.tensor.matmul(
                            pq[:, n2, :], lhsT=src_w[:, :, h, c, :],
                            rhs=ht[:, 2 * c:2 * c + 2,
                                   n * 512:(n + 1) * 512],
                            start=(c == 0), stop=(c == 3),
                            perf_mode=DR)
                nc.vector.tensor_scalar(
                    out=dst[:, h, 0, half * 1024:(half + 1) * 1024]
                    .rearrange("p (n m) -> p n m", n=2), in0=pq,
                    scalar1=1.0 / WSCALE, scalar2=bias[:, h:h + 1],
                    op0=mybir.AluOpType.mult, op1=mybir.AluOpType.add)

            def emit_qk(h):
                for u in range(4):
                    emit_qk_unit(h, u)

            def emit_st(h, q):
                """Scores + exp for one (head, q-chunk); returns E tiles."""
                ets = []
                for c in range(NKT // 2):
                    pst = ps_tile()
                    for i in range(2):
                        k = 2 * c + i
                        nc.tensor.matmul(
                            pst[:, i, :],
                            lhsT=kt[:, h, :, k * P:(k + 1) * P],
                            rhs=qt[:, h, :, q * 512:(q + 1) * 512],
                            start=True, stop=True, perf_mode=DR)
                    e_t = expp.tile([P, 2, 512], FP8, tag="e", name="e")
                    # shift by -6 so exp fits fp8e4 range (cancels in softmax)
                    nc.scalar.activation(out=e_t, in_=pst, func=AF.Exp,
                                         bias=nshift)
                    ets.append(e_t)
                return ets

            def emit_av(h, q, ets):
                """att@v + normalize for one (head, q-chunk)."""
                g = h // 4
                pot = ps_ot.tile([P, 512], F32, tag="ot", name="ot")
                for c in range(NKT // 2):
                    nc.tensor.matmul(pot, lhsT=vaug[c][:, :, h, :],
                                     rhs=ets[c],
                                     start=(c == 0), stop=(c == NKT // 2 - 1),
                                     perf_mode=DR)
                # rows 0:64 hold s/8 (const-block of 1/8 in vaug cols 0:64)
                rec_sb = att_sm.tile([HD, 512], F32, tag="rec", name="rec")
                nc.vector.reciprocal_approx_fast(out=rec_sb, in_=pot[0:HD, :])
                nc.vector.tensor_mul(
                    otn[g][64 * (h % 2):64 * (h % 2) + 64, (h % 4) // 2,
                           q * 512:(q + 1) * 512],
                    pot[HD:P, :], rec_sb)

            def emit_proj(t):
                """projection for one token tile (128 tokens x 1024 emb)."""
                for ec in range(2):
                    pz = ps_ot.tile([P, 512], F32, tag="ot", name="z")
                    for g in range(2):
                        nc.tensor.matmul(
                            pz, lhsT=otn[g][:, :, t * P:(t + 1) * P],
                            rhs=wp_s[:, :, g, ec * 512:(ec + 1) * 512],
                            start=(g == 0), stop=(g == 1), perf_mode=DR)
                    z_t = zst.tile([P, 512], BF16, tag="z", name="z")
                    nc.vector.tensor_scalar_mul(z_t, pz, 1.0 / (WSCALE * OSCALE))
                    nc.sync.dma_start(
                        out=z_d[t * P:(t + 1) * P, ec * 512:(ec + 1) * 512],
                        in_=z_t)

            # attention h-outer; QK of head h+1 woven into head h's stream;
            # projection woven into the last head's stream
            emit_qk(0)
            prev = None
            for h in range(HPC):
                for q in range(QCH):
                    ets = emit_st(h, q)
                    if prev is not None:
                        emit_av(*prev)
                        ph, pq_, _ = prev
                        if ph == HPC - 1 and pq_ > 0:
                            for t in range(4 * (pq_ - 1), 4 * pq_):
                                emit_proj(t)
                    prev = (h, q, ets)
                    if h + 1 < HPC:
                        emit_qk_unit(h + 1, q)
            emit_av(*prev)
            for t in range(4 * (QCH - 2), NT):
                emit_proj(t)


_CACHE = {}


def _get_nc():
    if "nc" not in _CACHE:
        _CACHE["nc"] = build_nc()
    return _CACHE["nc"]


def _prep_in_maps(x, ln_w, ln_b, w_qkv, b_qkv, w_proj, b_proj):
    bf = ml_dtypes.bfloat16
    f8 = ml_dtypes.float8_e4m3fn
    x = np.asarray(x, np.float32)
    ln_w = np.asarray(ln_w, np.float32)
    ln_b = np.asarray(ln_b, np.float32)
    w_qkv = np.asarray(w_qkv, np.float32)
    b_qkv = np.asarray(b_qkv, np.float32)
    w_proj = np.asarray(w_proj, np.float32)
    b_proj = np.asarray(b_proj, np.float32)

    b_eff = b_qkv + ln_b @ w_qkv
    w_eff = ln_w[:, None] * w_qkv
    w4 = w_eff.reshape(EMB, HEADS, HD, 3)
    b4 = b_eff.reshape(HEADS, HD, 3)
    wq = w4[..., 0] * SCALE
    wk = w4[..., 1]
    wv = w4[..., 2]
    bq = b4[..., 0] * SCALE
    bk = b4[..., 1]
    bv = b4[..., 2]

    def pack_qk(w, hsl):
        # w [EMB, 8 heads, 64] -> [128p, 2i, 8h, 4c, 64d] fp8 (x WSCALE)
        # emb = 256c + 128i + p
        wh = w[:, hsl, :]                                    # [1024, 8, 64]
        wh = wh.reshape(4, 2, P, HPC, HD)                    # c i p h d
        wh = wh.transpose(2, 1, 3, 0, 4)                     # p i h c d
        return np.ascontiguousarray((wh * WSCALE).reshape(P, -1)).astype(f8)

    def pack_qk_bias(b, hsl):
        # b [8 heads, 64] -> [64d, 8h] f32
        return np.ascontiguousarray(b[hsl].T.astype(np.float32))

    def pack_v(w, hsl):
        # w [EMB, 8, 64] -> [128p, 2i, 4c, 512hd] fp8 (x WSCALE)
        wh = w[:, hsl, :].reshape(4, 2, P, 512)              # c i p hd
        wh = wh.transpose(2, 1, 0, 3)                        # p i c hd
        return np.ascontiguousarray((wh * WSCALE).reshape(P, -1)).astype(f8)

    def pack_wp(w, hg):
        # w_proj rows for this head group [512, 1024] -> [128p, 2i, 2g, 1024]
        wh = w[hg * 512:(hg + 1) * 512, :]                   # hd=256g+128i+p
        wh = wh.reshape(2, 2, P, EMB).transpose(2, 1, 0, 3)  # p i g n
        return np.ascontiguousarray((wh * WSCALE).reshape(P, -1)).astype(f8)

    f8z = np.zeros((P, 8 * N_TOK), f8)
    vc = np.full((P, 2 * 8 * HD), 1.0 / OSCALE, f8)
    in_maps = []
    for cid in range(N_CORES):
        bi, hg = divmod(cid, 2)
        hsl = slice(hg * HPC, (hg + 1) * HPC)
        in_maps.append({
            "x": np.ascontiguousarray(x[bi]),
            "wq": pack_qk(wq, hsl),
            "wk": pack_qk(wk, hsl),
            "wv": pack_v(wv, hsl),
            "wp": pack_wp(w_proj, hg),
            "bqt": pack_qk_bias(bq, hsl),
            "bkt": pack_qk_bias(bk, hsl),
            "zpad": f8z,
            "vcst": vc,
        })
    return in_maps


def _gather(results, x, b_proj_eff):
    x = np.asarray(x, np.float32)
    out = np.empty((x.shape[0], N_TOK, EMB), np.float32)
    for bi in range(x.shape[0]):
        out[bi] = (results[2 * bi]["z"].astype(np.float32)
                   + results[2 * bi + 1]["z"].astype(np.float32)
                   + b_proj_eff[None, :] + x[bi])
    return out


def _run(inputs, **kw):
    in_maps = _prep_in_maps(**inputs)
    # exact fold of V bias into projection bias
    b_eff = np.asarray(inputs["b_qkv"], np.float32) + \
        np.asarray(inputs["ln_b"], np.float32) @ np.asarray(
            inputs["w_qkv"], np.float32)
    bv = b_eff.reshape(HEADS, HD, 3)[..., 2].reshape(HEADS * HD)
    b_proj_eff = np.asarray(inputs["b_proj"], np.float32) + \
        bv @ np.asarray(inputs["w_proj"], np.float32)
    res = run_bass_kernel_spmd(_get_nc(), in_maps,
                               core_ids=list(range(N_CORES)), **kw)
    out = _gather(res.results, inputs["x"], b_proj_eff)
    return out, res


def kernel(**inputs):
    out, _ = _run(inputs)
    return out


# revision 65
# speedup vs baseline: 1.7758x; 1.0240x over previous
"""Trainium2 Bass kernel for a pre-LN multi-head attention block.

Full-input contract: kernel(**inputs) takes the unsharded tensors from
setup_inputs() and returns the full [4, 2048, 1024] output.

Sharding: 8 cores = 4 batches x 2 head-groups (8 heads each).
Each core computes LayerNorm(x[b]), its 8 heads of QKV + attention, and a
partial projection.  Host sums the two partials per batch and adds
b_proj_eff + residual.

Numerics strategy: all matmuls run in fp8e4 (e4m3) DoubleRow perf mode
(2 contraction sub-rows per instruction at 0.5 cycles/row).  Weights are
pre-scaled x32 into fp8's representable range host-side; the 1/32 is
folded into the PSUM->SBUF cast passes.  The attention path is only ~8%
of the output norm (residual dominates), so fp8's ~6% element error
lands at ~2e-3 total relative error, well under tolerance.

Host-side algebraic folds (exact):
  - ln_w folded into w_qkv columns, ln_b folded into b_qkv
  - softmax scale folded into W_q / b_q
  - V bias folded into b_proj (attention rows sum to 1)

Layouts (DoubleRow pairs contraction rows along a middle dim of 2):
  ht    [128, 8, 2048] fp8: ht[p, 2c+i, t] = h[t, emb=256c+128i+p]
  qt/kt [g][128, 2, 2048] fp8: [32s+j, r, t] = Q[t, head 4g+s, d=32r+j]
  vaug  [kk][128, 2, 8, 65] fp8: [p, i, h, d] = V[tok=256kk+128i+p, h, d],
        d=64 column = 1.0 (softmax-sum row trick)
  E     [128, 2, 512] fp8 per (k-pair, q-chunk): [p, i, q]=exp(S[k,q])
  otn   [g][128, 2, 2048] fp8: [p, i, t] = 8*OT_norm[hd=256g+128i+p, t]
"""

import sys

sys.path.insert(0, "/opt/trn_rl_repo")

import numpy as np
import ml_dtypes

import concourse.bass as bass
from concourse import bacc
import concourse.tile as tile
from concourse import mybir
from concourse.bass_utils import run_bass_kernel_spmd
from concourse.masks import make_identity

EMB = 1024
HEADS = 16
HD = 64
SCALE = HD ** -0.5
N_TOK = 2048
N_CORES = 8
HPC = 8                 # heads per core
QK_COLS = HPC * HD      # 512
P = 128
NT = N_TOK // P         # 16 token tiles
QCH = 4                 # q chunks of 512
NKT = 16                # k tiles of 128
WSCALE = 32.0           # host pre-scale on all weights (fp8 range)
OSCALE = 8.0            # extra scale on normalized attn output

BF16 = mybir.dt.bfloat16
F32 = mybir.dt.float32
FP8 = mybir.dt.float8e4
AF = mybir.ActivationFunctionType
DR = mybir.MatmulPerfMode.DoubleRow


def build_nc():
    nc = bacc.Bacc(trn_type="TRN2", target_bir_lowering=False)

    x_d = nc.dram_tensor("x", [N_TOK, EMB], F32, kind="ExternalInput")
    wq_d = nc.dram_tensor("wq", [P, 2 * 8 * 4 * HD], FP8, kind="ExternalInput")
    wk_d = nc.dram_tensor("wk", [P, 2 * 8 * 4 * HD], FP8, kind="ExternalInput")
    wv_d = nc.dram_tensor("wv", [P, 2 * 4 * 512], FP8, kind="ExternalInput")
    wp_d = nc.dram_tensor("wp", [P, 2 * 2 * EMB], FP8, kind="ExternalInput")
    bqt_d = nc.dram_tensor("bqt", [HD, HPC], F32, kind="ExternalInput")
    bkt_d = nc.dram_tensor("bkt", [HD, HPC], F32, kind="ExternalInput")
    zpad_d = nc.dram_tensor("zpad", [P, 8 * N_TOK], FP8, kind="ExternalInput")
    vcst_d = nc.dram_tensor("vcst", [P, 2 * 8 * HD], FP8, kind="ExternalInput")
    z_d = nc.dram_tensor("z", [N_TOK, EMB], BF16, kind="ExternalOutput")

    with tile.TileContext(nc) as tc:
        _emit(nc, tc, x_d, wq_d, wk_d, wv_d, wp_d, bqt_d, bkt_d,
              zpad_d, vcst_d, z_d)
    nc.finalize()
    return nc


def _emit(nc, tc, x_d, wq_d, wk_d, wv_d, wp_d, bqt_d, bkt_d,
          zpad_d, vcst_d, z_d):
    from contextlib import ExitStack

    ctx = ExitStack()
    with ctx:
        consts = ctx.enter_context(tc.tile_pool(name="consts", bufs=1))
        persist = ctx.enter_context(tc.tile_pool(name="persist", bufs=1))

        eps_t = consts.tile([P, 1], F32, tag="eps", name="eps")
        nc.vector.memset(eps_t, 1e-5)
        nshift = consts.tile([P, 1], F32, tag="nshift", name="nshift")
        nc.vector.memset(nshift, -6.0)

        # weights: wq/wk as [128, 2(i), 8(h), 4(c), 64(m=d)]
        wq_s = persist.tile([P, 2, 8, 4, HD], FP8, tag="wq", name="wq")
        nc.sync.dma_start(out=wq_s, in_=wq_d[:, :].rearrange(
            "p (i h c m) -> p i h c m", i=2, h=8, c=4))
        wk_s = persist.tile([P, 2, 8, 4, HD], FP8, tag="wk", name="wk")
        nc.sync.dma_start(out=wk_s, in_=wk_d[:, :].rearrange(
            "p (i h c m) -> p i h c m", i=2, h=8, c=4))
        # wv as [128, 2(i), 4(c), 512(hd)]
        wv_s = persist.tile([P, 2, 4, 512], FP8, tag="wv", name="wv")
        nc.sync.dma_start(out=wv_s, in_=wv_d[:, :].rearrange(
            "p (i c m) -> p i c m", i=2, c=4))
        # wp as [128, 2(i), 2(g), 1024(n)]
        wp_s = persist.tile([P, 2, 2, EMB], FP8, tag="wp", name="wp")
        nc.sync.dma_start(out=wp_s, in_=wp_d[:, :].rearrange(
            "p (i g n) -> p i g n", i=2, g=2))
        bqt = consts.tile([HD, HPC], F32, tag="bqt", name="bqt")
        nc.sync.dma_start(out=bqt, in_=bqt_d[:, :])
        bkt = consts.tile([HD, HPC], F32, tag="bkt", name="bkt")
        nc.sync.dma_start(out=bkt, in_=bkt_d[:, :])

        ident = consts.tile([P, P], BF16, tag="ident", name="ident")
        make_identity(nc, ident)

        ht = persist.tile([P, 8, N_TOK], FP8, tag="ht", name="ht")
        # per-head Q^T/K^T at base partition 0 (DoubleRow needs row pos 0);
        # 64 partitions, DR slice i=1 zero-padded (halves ldweights rows)
        qt = persist.tile([HD, HPC, 2, N_TOK], FP8, tag="qt", name="qt")
        kt = persist.tile([HD, HPC, 2, N_TOK], FP8, tag="kt", name="kt")
        otn = [persist.tile([P, 2, N_TOK], FP8, tag=f"otn{g}", name=f"otn{g}")
               for g in range(2)]
        # cols 0:64 = 1/8, cols 64:128 = V: AV then leaves s/8 broadcast on
        # psum rows 0:64 (base 0), so normalize is approx-recip + mul
        vaug = [persist.tile([P, 2, HPC, P], FP8, tag=f"va{k}", name=f"va{k}")
                for k in range(NKT // 2)]
        # zero-pad / const fills via DMA on the Pool (SWDGE) queue so the
        # SP queue starts x tiles immediately
        nc.gpsimd.dma_start(out=qt[:, :, 1, :],
                            in_=zpad_d[0:HD, :].rearrange(
                                "p (h n) -> p h n", h=HPC))
        nc.gpsimd.dma_start(out=kt[:, :, 1, :],
                            in_=zpad_d[0:HD, :].rearrange(
                                "p (h n) -> p h n", h=HPC))
        for k in range(NKT // 2):
            nc.gpsimd.dma_start(out=vaug[k][:, :, :, 0:HD],
                                in_=vcst_d[:, :].rearrange(
                                    "p (i h d) -> p i h d", i=2, h=HPC))

        # ---- single shared PSUM pool for transposes/V/QK (via views) + ST --
        with tc.tile_pool(name="ln", bufs=5) as ln_pool, \
             tc.tile_pool(name="lns", bufs=3) as lns, \
             tc.tile_pool(name="expp", bufs=24) as expp, \
             tc.tile_pool(name="att_sm", bufs=6) as att_sm, \
             tc.tile_pool(name="zst", bufs=4) as zst, \
             tc.tile_pool(name="ps_st", bufs=3, space="PSUM") as ps_st, \
             tc.tile_pool(name="ps_ot", bufs=2, space="PSUM") as ps_ot:

            def ps_tile():
                return ps_st.tile([P, 2, 512], F32, tag="st", name="st")

            # ------------- Phase 1: LayerNorm + transpose + V -------------
            for nch in range(4):
                sd4 = lns.tile([P, 4], F32, tag="sd4", name="sd4")
                rstd4 = lns.tile([P, 4], F32, tag="rstd4", name="rstd4")
                nm4 = lns.tile([P, 4], F32, tag="nm4", name="nm4")
                mv4 = lns.tile([P, 4, 2], F32, tag="mv4", name="mv4")
                xts = []
                for t4 in range(4):
                    t = 4 * nch + t4
                    x_t = ln_pool.tile([P, EMB], F32, tag="x", name="x")
                    nc.sync.dma_start(out=x_t, in_=x_d[t * P:(t + 1) * P, :])
                    xts.append(x_t)
                    stats = lns.tile([P, 2, 6], F32, tag="stats", name="stats")
                    nc.vector.bn_stats(out=stats[:, 0, :], in_=x_t[:, 0:512])
                    nc.vector.bn_stats(out=stats[:, 1, :], in_=x_t[:, 512:1024])
                    nc.vector.bn_aggr(out=mv4[:, t4, :], in_=stats)
                    nc.scalar.activation(out=sd4[:, t4:t4 + 1],
                                         in_=mv4[:, t4, 1:2], func=AF.Sqrt,
                                         bias=eps_t, scale=1.0)
                nc.vector.reciprocal(out=rstd4, in_=sd4)
                nc.vector.tensor_scalar_mul(nm4, mv4[:, :, 0], -1.0)
                for t4 in range(4):
                    t = 4 * nch + t4
                    h_t = ln_pool.tile([P, EMB], BF16, tag="h", name="h")
                    nc.gpsimd.tensor_scalar(
                        out=h_t, in0=xts[t4],
                        scalar1=nm4[:, t4:t4 + 1], scalar2=rstd4[:, t4:t4 + 1],
                        op0=mybir.AluOpType.add, op1=mybir.AluOpType.mult)
                    pt = ps_tile().bitcast(BF16)[:, 0, :].rearrange(
                        "p (j m) -> p j m", j=8)
                    for j in range(8):
                        nc.tensor.transpose(pt[:, j, :],
                                            h_t[:, j * P:(j + 1) * P], ident)
                    nc.scalar.copy(out=ht[:, :, t * P:(t + 1) * P], in_=pt)
                    pv = ps_tile()[:, 0, :]
                    for c in range(4):
                        nc.tensor.matmul(
                            pv, lhsT=ht[:, 2 * c:2 * c + 2, t * P:(t + 1) * P],
                            rhs=wv_s[:, :, c, :],
                            start=(c == 0), stop=(c == 3), perf_mode=DR)
                    nc.vector.tensor_scalar_mul(
                        vaug[t // 2][:, t % 2, :, HD:P],
                        pv.rearrange("p (h d) -> p h d", h=HPC), 1.0 / WSCALE)

            # ---- QK for one head, one quarter (unit) at a time: woven into
            # the attention stream so ST production never pauses long ----
            def emit_qk_unit(h, u):
                src_w, bias, dst = ((wq_s, bqt, qt), (wk_s, bkt, kt))[u // 2]
                half = u % 2
                pq = ps_tile()[0:HD, :, :]
                for n2 in range(2):
                    n = 2 * half + n2
                    for c in range(4):
                        nc.tensor.matmul(
                            pq[:, n2, :], lhsT=src_w[:, :, h, c, :],
                            rhs=ht[:, 2 * c:2 * c + 2,
                                   n * 512:(n + 1) * 512],
                            start=(c == 0), stop=(c == 3),
                            perf_mode=DR)
                nc.vector.tensor_scalar(
                    out=dst[:, h, 0, half * 1024:(half + 1) * 1024]
                    .rearrange("p (n m) -> p n m", n=2), in0=pq,
                    scalar1=1.0 / WSCALE, scalar2=bias[:, h:h + 1],
                    op0=mybir.AluOpType.mult, op1=mybir.AluOpType.add)

            def emit_qk(h):
                for u in range(4):
                    emit_qk_unit(h, u)

            def emit_st(h, q):
                """Scores + exp for one (head, q-chunk); returns E tiles."""
                ets = []
                for c in range(NKT // 2):
                    pst = ps_tile()
                    for i in range(2):
                        k = 2 * c + i
                        nc.tensor.matmul(
                            pst[:, i, :],
                            lhsT=kt[:, h, :, k * P:(k + 1) * P],
                            rhs=qt[:, h, :, q * 512:(q + 1) * 512],
                            start=True, stop=True, perf_mode=DR)
                    e_t = expp.tile([P, 2, 512], FP8, tag="e", name="e")
                    # shift by -6 so exp fits fp8e4 range (cancels in softmax)
                    nc.scalar.activation(out=e_t, in_=pst, func=AF.Exp,
                                         bias=nshift)
                    ets.append(e_t)
                return ets

            def emit_av(h, q, ets):
                """att@v + normalize for one (head, q-chunk)."""
                g = h // 4
                pot = ps_ot.tile([P, 512], F32, tag="ot", name="ot")
                for c in range(NKT // 2):
                    nc.tensor.matmul(pot, lhsT=vaug[c][:, :, h, :],
                                     rhs=ets[c],
                                     start=(c == 0), stop=(c == NKT // 2 - 1),
                                     perf_mode=DR)
                # rows 0:64 hold s/8 (const-block of 1/8 in vaug cols 0:64)
                rec_sb = att_sm.tile([HD, 512], F32, tag="rec", name="rec")
                nc.vector.reciprocal_approx_fast(out=rec_sb, in_=pot[0:HD, :])
                nc.vector.tensor_mul(
                    otn[g][64 * (h % 2):64 * (h % 2) + 64, (h % 4) // 2,
                           q * 512:(q + 1) * 512],
                    pot[HD:P, :], rec_sb)

            def emit_proj(t):
                """projection for one token tile (128 tokens x 1024 emb)."""
                for ec in range(2):
                    pz = ps_ot.tile([P, 512], F32, tag="ot", name="z")
                    for g in range(2):
                        nc.tensor.matmul(
                            pz, lhsT=otn[g][:, :, t * P:(t + 1) * P],
                            rhs=wp_s[:, :, g, ec * 512:(ec + 1) * 512],
                            start=(g == 0), stop=(g == 1), perf_mode=DR)
                    z_t = zst.tile([P, 512], BF16, tag="z", name="z")
                    nc.vector.tensor_scalar_mul(z_t, pz, 1.0 / (WSCALE * OSCALE))
                    nc.sync.dma_start(
                        out=z_d[t * P:(t + 1) * P, ec * 512:(ec + 1) * 512],
                        in_=z_t)

            # attention h-outer; QK of head h+1 woven into head h's stream;
            # projection woven into the last head's stream
            emit_qk(0)
            prev = None
            for h in range(HPC):
                for q in range(QCH):
                    ets = emit_st(h, q)
                    if prev is not None:
                        emit_av(*prev)
                        ph, pq_, _ = prev
                        if ph == HPC - 1 and pq_ > 0:
                            for t in range(4 * (pq_ - 1), 4 * pq_):
                                emit_proj(t)
                    prev = (h, q, ets)
                    if h + 1 < HPC:
                        emit_qk_unit(h + 1, q)
            emit_av(*prev)
            for t in range(4 * (QCH - 2), NT):
                emit_proj(t)


_CACHE = {}


def _get_nc():
    if "nc" not in _CACHE:
        _CACHE["nc"] = build_nc()
    return _CACHE["nc"]


def _prep_in_maps(x, ln_w, ln_b, w_qkv, b_qkv, w_proj, b_proj):
    bf = ml_dtypes.bfloat16
    f8 = ml_dtypes.float8_e4m3fn
    x = np.asarray(x, np.float32)
    ln_w = np.asarray(ln_w, np.float32)
    ln_b = np.asarray(ln_b, np.float32)
    w_qkv = np.asarray(w_qkv, np.float32)
    b_qkv = np.asarray(b_qkv, np.float32)
    w_proj = np.asarray(w_proj, np.float32)
    b_proj = np.asarray(b_proj, np.float32)

    b_eff = b_qkv + ln_b @ w_qkv
    w_eff = ln_w[:, None] * w_qkv
    w4 = w_eff.reshape(EMB, HEADS, HD, 3)
    b4 = b_eff.reshape(HEADS, HD, 3)
    wq = w4[..., 0] * SCALE
    wk = w4[..., 1]
    wv = w4[..., 2]
    bq = b4[..., 0] * SCALE
    bk = b4[..., 1]
    bv = b4[..., 2]

    def pack_qk(w, hsl):
        # w [EMB, 8 heads, 64] -> [128p, 2i, 8h, 4c, 64d] fp8 (x WSCALE)
        # emb = 256c + 128i + p
        wh = w[:, hsl, :]                                    # [1024, 8, 64]
        wh = wh.reshape(4, 2, P, HPC, HD)                    # c i p h d
        wh = wh.transpose(2, 1, 3, 0, 4)                     # p i h c d
        return np.ascontiguousarray((wh * WSCALE).reshape(P, -1)).astype(f8)

    def pack_qk_bias(b, hsl):
        # b [8 heads, 64] -> [64d, 8h] f32
        return np.ascontiguousarray(b[hsl].T.astype(np.float32))

    def pack_v(w, hsl):
        # w [EMB, 8, 64] -> [128p, 2i, 4c, 512hd] fp8 (x WSCALE)
        wh = w[:, hsl, :].reshape(4, 2, P, 512)              # c i p hd
        wh = wh.transpose(2, 1, 0, 3)                        # p i c hd
        return np.ascontiguousarray((wh * WSCALE).reshape(P, -1)).astype(f8)

    def pack_wp(w, hg):
        # w_proj rows for this head group [512, 1024] -> [128p, 2i, 2g, 1024]
        wh = w[hg * 512:(hg + 1) * 512, :]                   # hd=256g+128i+p
        wh = wh.reshape(2, 2, P, EMB).transpose(2, 1, 0, 3)  # p i g n
        return np.ascontiguousarray((wh * WSCALE).reshape(P, -1)).astype(f8)

    f8z = np.zeros((P, 8 * N_TOK), f8)
    vc = np.full((P, 2 * 8 * HD), 1.0 / OSCALE, f8)
    in_maps = []
    for cid in range(N_CORES):
        bi, hg = divmod(cid, 2)
        hsl = slice(hg * HPC, (hg + 1) * HPC)
        in_maps.append({
            "x": np.ascontiguousarray(x[bi]),
            "wq": pack_qk(wq, hsl),
            "wk": pack_qk(wk, hsl),
            "wv": pack_v(wv, hsl),
            "wp": pack_wp(w_proj, hg),
            "bqt": pack_qk_bias(bq, hsl),
            "bkt": pack_qk_bias(bk, hsl),
            "zpad": f8z,
            "vcst": vc,
        })
    return in_maps


def _gather(results, x, b_proj_eff):
    x = np.asarray(x, np.float32)
    out = np.empty((x.shape[0], N_TOK, EMB), np.float32)
    for bi in range(x.shape[0]):
        out[bi] = (results[2 * bi]["z"].astype(np.float32)
                   + results[2 * bi + 1]["z"].astype(np.float32)
                   + b_proj_eff[None, :] + x[bi])
    return out


def _run(inputs, **kw):
    in_maps = _prep_in_maps(**inputs)
    # exact fold of V bias into projection bias
    b_eff = np.asarray(inputs["b_qkv"], np.float32) + \
        np.asarray(inputs["ln_b"], np.float32) @ np.asarray(
            inputs["w_qkv"], np.float32)
    bv = b_eff.reshape(HEADS, HD, 3)[..., 2].reshape(HEADS * HD)
    b_proj_eff = np.asarray(inputs["b_proj"], np.float32) + \
        bv @ np.asarray(inputs["w_proj"], np.float32)
    res = run_bass_kernel_spmd(_get_nc(), in_maps,
                               core_ids=list(range(N_CORES)), **kw)
    out = _gather(res.results, inputs["x"], b_proj_eff)
    return out, res


def kernel(**inputs):
    out, _ = _run(inputs)
    return out
